# revision 3
# baseline (speedup 1.0000x reference)
"""DeepTypedGraphNet (GNN message passing) Trainium2 kernel v2, 8-core SPMD.

Design (vs v1 baseline): the whole program runs in hardware For_i loops with
affine addressing, cutting the BIR from ~100k to ~2k instructions (host build
+ rust compile + NEFF all shrink ~50x). Edges are packed per receiver-window:
core k owns nodes [12500k, 12500(k+1)), padded to 12800 = 100 windows of 128
nodes; each window gets 1024 edge slots = 8 chunks of 128, chunk j serving
sender-bank j//2 (4 banks of 25600 padded-global rows keep dma_gather's int16
index range). Receiver latents are expanded per chunk with a one-hot C matrix
(built on device from iota + is_equal); segment-sum aggregation is the same C
matrix used as matmul lhsT accumulating in PSUM -- no scatter, no agg tables.
Everything lives in latent-major (T) layout; LayerNorm reduces over latent =
partitions via ones-matmul column sums that broadcast for free. Sender latents
come from an AllGathered padded node table via dma_gather(transpose=True).
"""
import sys
sys.path.insert(0, '/opt/trn_rl_repo')

import numpy as np
import ml_dtypes

import concourse.bacc as bacc
import concourse.mybir as mybir
import concourse.tile as tile
from concourse.bass import ds, ts

BF16 = ml_dtypes.bfloat16
F32 = np.float32

LN_EPS = 1e-5
LATENT = 256
HIDDEN = 256
D_NODE_IN = 128
D_EDGE_IN = 4
D_OUT = 128
STEPS = 6

NC = 8
CHUNK_REAL = 12500
CHUNK = 12800            # 100 windows * 128
NW = CHUNK // 128        # 100 receiver windows per core
TAB = NC * CHUNK         # 102400 padded global nodes
NBANK = 4
BANK = TAB // NBANK      # 25600 (int16-safe gather range)
QUOTA = 256              # slots per (window, bank) = 2 chunks
WSLOTS = NBANK * QUOTA   # 1024 slots per window
ES = NW * WSLOTS         # 102400 edge slots per core
NT = CHUNK // 512        # 25 node pieces of 512

# blob layouts: (name, tile_shape); column count = prod(shape[1:])
BFSPEC = [
    ('nfT', (128, CHUNK)), ('ident', (128, 128)),
    ('enc_n_w1', (128, 1, 256)), ('enc_n_b1', (128, 2)),
    ('enc_n_w2', (128, 2, 256)), ('enc_n_b2', (128, 2)),
    ('enc_e_b1', (128, 2)), ('enc_e_w2', (128, 2, 256)), ('enc_e_b2', (128, 2)),
    ('pe_w1', (128, 6 * STEPS, 256)), ('pe_b1', (128, 2 * STEPS)),
    ('pe_w2', (128, 2 * STEPS, 256)), ('pe_b2', (128, 2 * STEPS)),
    ('pn_w1', (128, 4 * STEPS, 256)), ('pn_b1', (128, 2 * STEPS)),
    ('pn_w2', (128, 2 * STEPS, 256)), ('pn_b2', (128, 2 * STEPS)),
    ('dec_w1', (128, 2, 256)), ('dec_b1', (128, 2)),
    ('dec_w2', (128, 2, 128)), ('dec_b2', (128, 1)),
]
F32SPEC = [
    ('iota', (128, 128)), ('rcvc', (128, ES // 128)),
    ('enc_n_s', (128, 2)), ('enc_n_o', (128, 2)),
    ('enc_e_s', (128, 2)), ('enc_e_o', (128, 2)),
    ('pe_s', (128, 2 * STEPS)), ('pe_o', (128, 2 * STEPS)),
    ('pn_s', (128, 2 * STEPS)), ('pn_o', (128, 2 * STEPS)),
]


def _blob_offsets(spec):
    off = {}
    c = 0
    for name, shape in spec:
        n = int(np.prod(shape[1:]))
        off[name] = (c, n, shape)
        c += n
    return off, c


BF_OFF, BF_COLS = _blob_offsets(BFSPEC)
F32_OFF, F32_COLS = _blob_offsets(F32SPEC)


# ----------------------------------------------------------------------------
# host-side prep
# ----------------------------------------------------------------------------

def _wrap_idx(vals):
    """[n] int16 -> [16, n/16] wrapped: slot i at [i%16, i//16]. The required
    replication across the 8 groups of 16 partitions happens on device."""
    n = len(vals)
    a = np.asarray(vals, np.int16).reshape(n // 16, 16).T
    return np.ascontiguousarray(a)


def _prep_graph(senders, receivers, edge_features):
    s = np.asarray(senders, np.int64)
    r = np.asarray(receivers, np.int64)
    ef = np.asarray(edge_features, F32)
    E = len(s)
    ks = s // CHUNK_REAL
    ps = ks * CHUNK + (s - ks * CHUNK_REAL)      # padded global sender id
    kr = r // CHUNK_REAL
    rl = r - kr * CHUNK_REAL                     # local receiver id
    w = rl // 128
    pos = rl % 128
    b = ps // BANK

    group = ((kr * NW + w) * NBANK + b)
    order = np.argsort(group, kind='stable')
    g_sorted = group[order]
    # index within group
    uniq, starts, counts = np.unique(g_sorted, return_index=True, return_counts=True)
    if counts.max() > QUOTA:
        raise RuntimeError(f"(window,bank) overflow: {counts.max()} > {QUOTA}")
    within = np.arange(E) - np.repeat(starts, counts)
    slot = g_sorted * QUOTA + within              # global slot id: core*ES + local
    core = g_sorted // (NW * NBANK)
    local_slot = slot - core * (NW * NBANK * QUOTA)

    out = []
    for k in range(NC):
        sel = core == k
        e_ids = order[sel]
        sl = local_slot[sel]
        snd_rel = np.zeros(ES, np.int16)
        rcv_val = np.full(ES, 255.0, F32)
        efT = np.zeros((D_EDGE_IN, ES), F32)
        snd_rel[sl] = (ps[e_ids] - b[e_ids] * BANK).astype(np.int16)
        rcv_val[sl] = pos[e_ids].astype(F32)
        efT[:, sl] = ef[e_ids].T
        rcv_col = np.ascontiguousarray(rcv_val.reshape(ES // 128, 128).T)  # [128, 800]
        eid = np.full(ES, -1, np.int64)
        eid[sl] = e_ids
        out.append(dict(snd=_wrap_idx(snd_rel), rcvc=rcv_col,
                        efT=efT.astype(BF16), _eid=eid))
    return out


def _prep_weights(i):
    """Pack weights into the kernel layouts (bf16 unless noted)."""
    w = {}

    def w1pack(mat):  # [K, N] -> [128, K/128, N] lhsT chunks
        K, N = mat.shape
        assert K % 128 == 0
        return np.ascontiguousarray(
            np.asarray(mat, F32).reshape(K // 128, 128, N).transpose(1, 0, 2))

    def cols(vec_s):  # [S, 256] -> [128, S*2]
        v = np.asarray(vec_s, F32)
        if v.ndim == 1:
            v = v[None]
        S = v.shape[0]
        return np.ascontiguousarray(v.reshape(S, 2, 128).transpose(2, 0, 1).reshape(128, S * 2))

    w['enc_n_w1'] = w1pack(i['enc_node_w1'])                       # [128,1,256]
    w['enc_n_b1'] = cols(i['enc_node_b1'])
    w['enc_n_w2'] = w1pack(i['enc_node_w2'])                       # [128,2,256]
    w['enc_n_b2'] = cols(i['enc_node_b2'])
    w['enc_n_s'] = cols(i['enc_node_ln_s'])
    w['enc_n_o'] = cols(i['enc_node_ln_o'])
    w['enc_e_w1'] = np.asarray(i['enc_edge_w1'], F32)[:, None, :]  # [4,1,256]
    w['enc_e_b1'] = cols(i['enc_edge_b1'])
    w['enc_e_w2'] = w1pack(i['enc_edge_w2'])
    w['enc_e_b2'] = cols(i['enc_edge_b2'])
    w['enc_e_s'] = cols(i['enc_edge_ln_s'])
    w['enc_e_o'] = cols(i['enc_edge_ln_o'])

    w['pe_w1'] = np.concatenate([w1pack(i['pe_w1'][s]) for s in range(STEPS)], 1)  # [128,36,256]
    w['pe_b1'] = cols(i['pe_b1'])
    w['pe_w2'] = np.concatenate([w1pack(i['pe_w2'][s]) for s in range(STEPS)], 1)  # [128,12,256]
    w['pe_b2'] = cols(i['pe_b2'])
    w['pe_s'] = cols(i['pe_ln_s'])
    w['pe_o'] = cols(i['pe_ln_o'])
    w['pn_w1'] = np.concatenate([w1pack(i['pn_w1'][s]) for s in range(STEPS)], 1)  # [128,24,256]
    w['pn_b1'] = cols(i['pn_b1'])
    w['pn_w2'] = np.concatenate([w1pack(i['pn_w2'][s]) for s in range(STEPS)], 1)
    w['pn_b2'] = cols(i['pn_b2'])
    w['pn_s'] = cols(i['pn_ln_s'])
    w['pn_o'] = cols(i['pn_ln_o'])

    w['dec_w1'] = w1pack(i['dec_w1'])                              # [128,2,256]
    w['dec_b1'] = cols(i['dec_b1'])
    w['dec_w2'] = w1pack(np.asarray(i['dec_w2'], F32))             # [128,2,128]
    w['dec_b2'] = np.asarray(i['dec_b2'], F32)[:, None]            # [128,1]

    w['iota'] = np.tile(np.arange(128, dtype=F32)[None, :], (128, 1))
    w['ident'] = np.eye(128, dtype=F32)

    # pack into blobs (nfT / rcvc regions are per-core, filled by caller)
    blob_bf = np.zeros((128, BF_COLS), BF16)
    for name, (c0, n, shape) in BF_OFF.items():
        if name == 'nfT':
            continue
        blob_bf[:, c0:c0 + n] = w[name].reshape(128, n).astype(BF16)
    blob_f32 = np.zeros((128, F32_COLS), F32)
    for name, (c0, n, shape) in F32_OFF.items():
        if name == 'rcvc':
            continue
        blob_f32[:, c0:c0 + n] = w[name].reshape(128, n).astype(F32)
    return blob_bf, blob_f32, np.ascontiguousarray(w['enc_e_w1'].astype(BF16))


# ----------------------------------------------------------------------------
# program
# ----------------------------------------------------------------------------

def build_program(steps=STEPS, debug=False):
    dt = mybir.dt
    bf = dt.bfloat16
    f32 = dt.float32
    AF = mybir.ActivationFunctionType
    OP = mybir.AluOpType

    nc = bacc.Bacc(None, target_bir_lowering=False)

    def inp(name, shape, dtype=bf):
        return nc.dram_tensor(name, shape, dtype, kind="ExternalInput")

    t_bf = inp("blob_bf", [128, BF_COLS])
    t_f32 = inp("blob_f32", [128, F32_COLS], f32)
    t_efT = inp("efT", [D_EDGE_IN, ES])
    t_snd = inp("snd16", [16, ES // 16], dt.int16)
    t_eew1 = inp("enc_e_w1", [D_EDGE_IN, 1, 256])

    t_out = nc.dram_tensor("outp", [CHUNK, D_OUT], bf, kind="ExternalOutput")

    # internal DRAM
    node_loc = nc.dram_tensor("node_loc", [CHUNK, LATENT], bf)
    cc_out = nc.dram_tensor("cc_out", [TAB, LATENT], bf, addr_space="Shared")
    edge_T = nc.dram_tensor("edge_T", [128, 2, ES], bf)
    nodeT = nc.dram_tensor("nodeT", [128, 2, CHUNK], bf)
    aggT = nc.dram_tensor("aggT", [128, 2, CHUNK], bf)
    C_d = nc.dram_tensor("C_d", [128, ES], bf)
    CT_d = nc.dram_tensor("CT_d", [128, ES], bf)
    snd_x = nc.dram_tensor("snd_x", [128, ES // 16], dt.int16)

    with tile.TileContext(nc) as tc:
        import contextlib
        stack = contextlib.ExitStack()
        wp = stack.enter_context(tc.tile_pool(name="wp", bufs=1))
        sb = stack.enter_context(tc.tile_pool(name="sb", bufs=2))
        ps2 = stack.enter_context(tc.tile_pool(name="ps2", bufs=2, space="PSUM"))
        ps1 = stack.enter_context(tc.tile_pool(name="ps1", bufs=1, space="PSUM"))

        wt = {}
        for name, (c0, ncols, shape) in list(BF_OFF.items()) + list(F32_OFF.items()):
            if name in ('nfT', 'rcvc'):
                continue
            if name in dict(F32SPEC):
                blob, dtype = t_f32, f32
            else:
                blob, dtype = t_bf, bf
            t = wp.tile(list(shape), dtype, tag=f"w_{name}")
            src = blob.ap()[:, c0:c0 + ncols]
            if len(shape) == 3:
                src = src.rearrange("p (k n) -> p k n", k=shape[1])
            nc.sync.dma_start(out=t[:], in_=src)
            wt[name] = t
        t = wp.tile([D_EDGE_IN, 1, 256], bf, tag="w_enc_e_w1")
        nc.sync.dma_start(out=t[:], in_=t_eew1[:])
        wt['enc_e_w1'] = t
        ones_t = wp.tile([128, 128], bf, tag="ones")
        nc.vector.memset(ones_t[:], 1.0)
        eps_t = wp.tile([128, 1], f32, tag="eps")
        nc.vector.memset(eps_t[:], LN_EPS)

        # expand snd16 [16, ES/16] -> snd_x [128, ES/16] (8 Q7-core replicas)
        s16 = wp.tile([16, ES // 16], dt.int16, tag="s16")
        nc.sync.dma_start(out=s16[:], in_=t_snd[:])
        for g in range(8):
            nc.sync.dma_start(out=snd_x.ap()[16 * g:16 * g + 16, :], in_=s16[:])

        def ln_half(o2, kb2, ks, ko, resid_ap, out_ap, fw):
            """PSUM o2 [128,2,FW] f32 -> LayerNorm over latent(=partitions,
            2 chunks) with scale/offset cols, optional residual, bf16 out."""
            o2sb = sb.tile([128, 2, 512], f32, tag="o2sb")
            o2bf = sb.tile([128, 2, 512], bf, tag="o2bf")
            sq = sb.tile([128, 2, 512], bf, tag="sq")
            for lc in range(2):
                nc.scalar.activation(out=o2sb[:, lc, :fw], in_=o2[:, lc, :fw],
                                     func=AF.Identity, bias=kb2[:, lc:lc + 1])
                nc.vector.tensor_copy(out=o2bf[:, lc, :fw], in_=o2sb[:, lc, :fw])
                nc.scalar.activation(out=sq[:, lc, :fw], in_=o2sb[:, lc, :fw],
                                     func=AF.Square)
            S = ps1.tile([128, 512], f32, tag="S")
            nc.tensor.matmul(S[:, :fw], lhsT=ones_t[:], rhs=o2bf[:, 0, :fw],
                             start=True, stop=False)
            nc.tensor.matmul(S[:, :fw], lhsT=ones_t[:], rhs=o2bf[:, 1, :fw],
                             start=False, stop=True)
            S2 = ps1.tile([128, 512], f32, tag="S2")
            nc.tensor.matmul(S2[:, :fw], lhsT=ones_t[:], rhs=sq[:, 0, :fw],
                             start=True, stop=False)
            nc.tensor.matmul(S2[:, :fw], lhsT=ones_t[:], rhs=sq[:, 1, :fw],
                             start=False, stop=True)
            m = sb.tile([128, 512], f32, tag="st_m")
            v = sb.tile([128, 512], f32, tag="st_v")
            nc.vector.tensor_scalar(out=m[:, :fw], in0=S[:, :fw], scalar1=1.0 / LATENT,
                                    scalar2=None, op0=OP.mult)
            nc.vector.tensor_scalar(out=v[:, :fw], in0=S2[:, :fw], scalar1=1.0 / LATENT,
                                    scalar2=None, op0=OP.mult)
            msq = sb.tile([128, 512], f32, tag="st_m2")
            nc.vector.tensor_tensor(out=msq[:, :fw], in0=m[:, :fw], in1=m[:, :fw],
                                    op=OP.mult)
            nc.vector.tensor_tensor(out=v[:, :fw], in0=v[:, :fw], in1=msq[:, :fw],
                                    op=OP.subtract)
            inv = sb.tile([128, 512], f32, tag="st_i")
            nc.scalar.activation(out=inv[:, :fw], in_=v[:, :fw], func=AF.Sqrt,
                                 bias=eps_t[:])
            nc.vector.reciprocal(inv[:, :fw], inv[:, :fw])
            b = sb.tile([128, 512], f32, tag="st_b")
            nc.vector.tensor_tensor(out=b[:, :fw], in0=m[:, :fw], in1=inv[:, :fw],
                                    op=OP.mult)
            for lc in range(2):
                y = sb.tile([128, 512], f32, tag="st_y")
                nc.vector.tensor_tensor(out=y[:, :fw], in0=o2sb[:, lc, :fw],
                                        in1=inv[:, :fw], op=OP.mult)
                nc.vector.tensor_tensor(out=y[:, :fw], in0=y[:, :fw], in1=b[:, :fw],
                                        op=OP.subtract)
                z = sb.tile([128, 512], f32, tag="st_z")
                nc.vector.tensor_scalar(out=z[:, :fw], in0=y[:, :fw],
                                        scalar1=ks[:, lc:lc + 1],
                                        scalar2=ko[:, lc:lc + 1],
                                        op0=OP.mult, op1=OP.add)
                if resid_ap is not None:
                    nc.vector.tensor_tensor(out=out_ap[lc], in0=z[:, :fw],
                                            in1=resid_ap[lc], op=OP.add)
                else:
                    nc.vector.tensor_copy(out=out_ap[lc], in_=z[:, :fw])

        # ---------------- C build ----------------
        with tc.For_i(0, NW, name="cbuild") as w:
            rc = sb.tile([128, 8], f32, tag="rc")
            nc.sync.dma_start(out=rc[:], in_=t_f32[:, ds(w * 8 + F32_OFF['rcvc'][0], 8)])
            cw = sb.tile([128, WSLOTS], bf, tag="cw")
            ctw = sb.tile([128, WSLOTS], bf, tag="ctw")
            for j in range(8):
                nc.vector.tensor_scalar(out=cw[:, 128 * j:128 * j + 128],
                                        in0=wt['iota'][:], scalar1=rc[:, j:j + 1],
                                        scalar2=None, op0=OP.is_equal)
                tp = ps2.tile([128, 128], bf, tag="sm")
                nc.tensor.transpose(out=tp[:], in_=cw[:, 128 * j:128 * j + 128],
                                    identity=wt['ident'][:])
                if j % 2 == 0:
                    nc.vector.tensor_copy(out=ctw[:, 128 * j:128 * j + 128], in_=tp[:])
                else:
                    nc.scalar.activation(out=ctw[:, 128 * j:128 * j + 128], in_=tp[:],
                                         func=AF.Copy)
            nc.sync.dma_start(out=C_d[:, ts(w, WSLOTS)], in_=cw[:])
            nc.sync.dma_start(out=CT_d[:, ts(w, WSLOTS)], in_=ctw[:])

        # ---------------- node encoder ----------------
        def mlp_tail(htb, wname, bname, sname, oname, sidx, resid, outf, t, dram,
                     fw=512):
            """L2 (+bias) + LN + optional residual + store to dram[:, :, ts]."""
            o2 = ps1.tile([128, 2, 512], f32, tag="o2")
            for lc in range(2):
                for kc in range(2):
                    nc.tensor.matmul(o2[:, lc, :fw],
                                     lhsT=wt[wname][:, 2 * sidx + kc, 128 * lc:128 * lc + 128],
                                     rhs=htb[:, kc, :fw],
                                     start=(kc == 0), stop=(kc == 1))
            newt = sb.tile([128, 2, 512], bf, tag="newt")
            resid_ap = None
            if resid is not None:
                resid_ap = [resid[:, lc, :fw] for lc in range(2)]
            ln_half(o2, wt[bname][:, 2 * sidx:2 * sidx + 2],
                    wt[sname][:, 2 * sidx:2 * sidx + 2],
                    wt[oname][:, 2 * sidx:2 * sidx + 2],
                    resid_ap, [newt[:, lc, :fw] for lc in range(2)], fw)
            nc.sync.dma_start(out=dram[:, :, ts(t, fw)], in_=newt[:, :, :fw])
            return newt

        def store_normal(newt, t, dram, nch=4, dtile=LATENT):
            """Transpose [128,2,512]T tile -> normal rows, write dram rows."""
            nn = sb.tile([128, 4, 256], dram.dtype, tag="nn4")
            for c in range(nch):
                for lc in range(2):
                    tp = ps2.tile([128, 128], bf, tag="sm")
                    nc.tensor.transpose(out=tp[:], in_=newt[:, lc, 128 * c:128 * c + 128],
                                        identity=wt['ident'][:])
                    if (c + lc) % 2 == 0:
                        nc.vector.tensor_copy(out=nn[:, c, 128 * lc:128 * lc + 128], in_=tp[:])
                    else:
                        nc.scalar.activation(out=nn[:, c, 128 * lc:128 * lc + 128],
                                             in_=tp[:], func=AF.Copy)
            nc.sync.dma_start(
                out=dram.ap()[ts(t, 512)].rearrange("(c p) d -> p c d", p=128),
                in_=nn[:, :nch, :])

        with tc.For_i(0, NT, name="enc_n") as t:
            nf = sb.tile([128, 512], bf, tag="nf")
            nc.sync.dma_start(out=nf[:], in_=t_bf[:, ds(t * 512 + BF_OFF['nfT'][0], 512)])
            htb = sb.tile([128, 2, 512], bf, tag="htb")
            for m in range(2):
                hp = ps1.tile([128, 512], f32, tag="hp")
                nc.tensor.matmul(hp[:], lhsT=wt['enc_n_w1'][:, 0, 128 * m:128 * m + 128],
                                 rhs=nf[:], start=True, stop=True)
                nc.scalar.activation(out=htb[:, m, :], in_=hp[:], func=AF.Silu,
                                     bias=wt['enc_n_b1'][:, m:m + 1])
            newt = mlp_tail(htb, 'enc_n_w2', 'enc_n_b2', 'enc_n_s', 'enc_n_o', 0,
                            None, None, t, nodeT)
            store_normal(newt, t, node_loc)

        # ---------------- edge encoder ----------------
        with tc.For_i(0, NW, name="enc_e") as w:
            ef = sb.tile([D_EDGE_IN, WSLOTS], bf, tag="ef")
            nc.sync.dma_start(out=ef[:], in_=t_efT[:, ts(w, WSLOTS)])
            for h in range(2):
                htb = sb.tile([128, 2, 512], bf, tag="htb")
                for m in range(2):
                    hp = ps1.tile([128, 512], f32, tag="hp")
                    nc.tensor.matmul(hp[:], lhsT=wt['enc_e_w1'][:, 0, 128 * m:128 * m + 128],
                                     rhs=ef[:, 512 * h:512 * h + 512], start=True, stop=True)
                    nc.scalar.activation(out=htb[:, m, :], in_=hp[:], func=AF.Silu,
                                         bias=wt['enc_e_b1'][:, m:m + 1])
                o2 = ps1.tile([128, 2, 512], f32, tag="o2")
                for lc in range(2):
                    for kc in range(2):
                        nc.tensor.matmul(o2[:, lc, :],
                                         lhsT=wt['enc_e_w2'][:, kc, 128 * lc:128 * lc + 128],
                                         rhs=htb[:, kc, :], start=(kc == 0), stop=(kc == 1))
                newt = sb.tile([128, 2, 512], bf, tag="newt")
                ln_half(o2, wt['enc_e_b2'][:, 0:2], wt['enc_e_s'][:, 0:2],
                        wt['enc_e_o'][:, 0:2], None,
                        [newt[:, lc, :] for lc in range(2)], 512)
                nc.sync.dma_start(out=edge_T[:, :, ds(w * WSLOTS + 512 * h, 512)],
                                  in_=newt[:])

        # ---------------- message passing ----------------
        for s in range(steps):
            nc.gpsimd.collective_compute(
                "AllGather", mybir.AluOpType.bypass,
                ins=[node_loc[:]], outs=[cc_out[:]],
                replica_groups=[list(range(NC))])

            with tc.For_i(0, NW, name=f"edge{s}") as w:
                xe = sb.tile([128, 2, WSLOTS], bf, tag="xe")
                nc.sync.dma_start(out=xe[:], in_=edge_T[:, :, ts(w, WSLOTS)])
                idxt = sb.tile([128, WSLOTS // 16], mybir.dt.int16, tag="idxt")
                nc.sync.dma_start(out=idxt[:], in_=snd_x.ap()[:, ts(w, WSLOTS // 16)])
                xst = sb.tile([128, NBANK, 2, QUOTA], bf, tag="xst")
                for b in range(NBANK):
                    nc.gpsimd.dma_gather(
                        out_ap=xst[:, b, :, :],
                        in_ap=cc_out.ap()[BANK * b:BANK * b + BANK],
                        idxs_ap=idxt[:, 16 * b:16 * b + 16],
                        num_idxs=QUOTA, num_idxs_reg=QUOTA, elem_size=LATENT,
                        transpose=True)
                xs = sb.tile([128, 2, WSLOTS], bf, tag="xs")
                for b in range(NBANK):
                    for lc in range(2):
                        if (b + lc) % 2 == 0:
                            nc.vector.tensor_copy(out=xs[:, lc, QUOTA * b:QUOTA * b + QUOTA],
                                                  in_=xst[:, b, lc, :])
                        else:
                            nc.scalar.activation(out=xs[:, lc, QUOTA * b:QUOTA * b + QUOTA],
                                                 in_=xst[:, b, lc, :], func=AF.Copy)
                ndw = sb.tile([128, LATENT], bf, tag="ndw")
                nc.sync.dma_start(out=ndw[:], in_=node_loc.ap()[ts(w, 128)])
                ct = sb.tile([128, WSLOTS], bf, tag="ct")
                nc.sync.dma_start(out=ct[:], in_=CT_d[:, ts(w, WSLOTS)])
                cc = sb.tile([128, WSLOTS], bf, tag="cc")
                nc.sync.dma_start(out=cc[:], in_=C_d[:, ts(w, WSLOTS)])
                xr = sb.tile([128, 2, WSLOTS], bf, tag="xr")
                for j in range(8):
                    for lc in range(2):
                        xp = ps2.tile([128, 128], f32, tag="sm")
                        nc.tensor.matmul(xp[:], lhsT=ndw[:, 128 * lc:128 * lc + 128],
                                         rhs=ct[:, 128 * j:128 * j + 128],
                                         start=True, stop=True)
                        if (j + lc) % 2 == 0:
                            nc.vector.tensor_copy(out=xr[:, lc, 128 * j:128 * j + 128], in_=xp[:])
                        else:
                            nc.scalar.activation(out=xr[:, lc, 128 * j:128 * j + 128],
                                                 in_=xp[:], func=AF.Copy)

                newe = sb.tile([128, 2, WSLOTS], bf, tag="newe")
                for h in range(2):
                    htb = sb.tile([128, 2, 512], bf, tag="htb")
                    for m in range(2):
                        hp = ps1.tile([128, 512], f32, tag="hp")
                        first = True
                        for src, kbase in ((xe, 0), (xs, 2), (xr, 4)):
                            for kc in range(2):
                                ci = kbase + kc
                                nc.tensor.matmul(
                                    hp[:], lhsT=wt['pe_w1'][:, 6 * s + ci, 128 * m:128 * m + 128],
                                    rhs=src[:, kc, 512 * h:512 * h + 512],
                                    start=first, stop=(ci == 5))
                                first = False
                        nc.scalar.activation(out=htb[:, m, :], in_=hp[:], func=AF.Silu,
                                             bias=wt['pe_b1'][:, 2 * s + m:2 * s + m + 1])
                    o2 = ps1.tile([128, 2, 512], f32, tag="o2")
                    for lc in range(2):
                        for kc in range(2):
                            nc.tensor.matmul(o2[:, lc, :],
                                             lhsT=wt['pe_w2'][:, 2 * s + kc, 128 * lc:128 * lc + 128],
                                             rhs=htb[:, kc, :], start=(kc == 0), stop=(kc == 1))
                    ln_half(o2, wt['pe_b2'][:, 2 * s:2 * s + 2],
                            wt['pe_s'][:, 2 * s:2 * s + 2], wt['pe_o'][:, 2 * s:2 * s + 2],
                            [xe[:, lc, 512 * h:512 * h + 512] for lc in range(2)],
                            [newe[:, lc, 512 * h:512 * h + 512] for lc in range(2)], 512)
                nc.sync.dma_start(out=edge_T[:, :, ts(w, WSLOTS)], in_=newe[:])

                # aggregation: aggT[:, lc, w*128:] = sum_j newe_j^T @ C_j
                # (the two lc chains share one PSUM bank: keep each chain
                # contiguous -- a group's first matmul clears the whole
                # bank's has_written bits)
                agg = ps1.tile([128, 2, 128], f32, tag="agg")
                nn8 = sb.tile([128, 8, 256], bf, tag="nn8")
                for j in range(8):
                    for lc in range(2):
                        tp = ps2.tile([128, 128], bf, tag="sm")
                        nc.tensor.transpose(out=tp[:], in_=newe[:, lc, 128 * j:128 * j + 128],
                                            identity=wt['ident'][:])
                        if lc == 0:
                            nc.vector.tensor_copy(out=nn8[:, j, 128 * lc:128 * lc + 128], in_=tp[:])
                        else:
                            nc.scalar.activation(out=nn8[:, j, 128 * lc:128 * lc + 128],
                                                 in_=tp[:], func=AF.Copy)
                for lc in range(2):
                    for j in range(8):
                        nc.tensor.matmul(agg[:, lc, :], lhsT=nn8[:, j, 128 * lc:128 * lc + 128],
                                         rhs=cc[:, 128 * j:128 * j + 128],
                                         start=(j == 0), stop=(j == 7))
                agsb = sb.tile([128, 2, 128], bf, tag="agsb")
                nc.vector.tensor_copy(out=agsb[:, 0, :], in_=agg[:, 0, :])
                nc.scalar.activation(out=agsb[:, 1, :], in_=agg[:, 1, :], func=AF.Copy)
                nc.sync.dma_start(out=aggT[:, :, ts(w, 128)], in_=agsb[:])

            with tc.For_i(0, NT, name=f"node{s}") as t:
                ntw = sb.tile([128, 2, 512], bf, tag="ntw")
                nc.sync.dma_start(out=ntw[:], in_=nodeT[:, :, ts(t, 512)])
                agw = sb.tile([128, 2, 512], bf, tag="agw")
                nc.sync.dma_start(out=agw[:], in_=aggT[:, :, ts(t, 512)])
                htb = sb.tile([128, 2, 512], bf, tag="htb")
                for m in range(2):
                    hp = ps1.tile([128, 512], f32, tag="hp")
                    first = True
                    for src, kbase in ((ntw, 0), (agw, 2)):
                        for kc in range(2):
                            ci = kbase + kc
                            nc.tensor.matmul(
                                hp[:], lhsT=wt['pn_w1'][:, 4 * s + ci, 128 * m:128 * m + 128],
                                rhs=src[:, kc, :], start=first, stop=(ci == 3))
                            first = False
                    nc.scalar.activation(out=htb[:, m, :], in_=hp[:], func=AF.Silu,
                                         bias=wt['pn_b1'][:, 2 * s + m:2 * s + m + 1])
                newt = mlp_tail(htb, 'pn_w2', 'pn_b2', 'pn_s', 'pn_o', s, ntw,
                                None, t, nodeT)
                store_normal(newt, t, node_loc)

        # ---------------- decoder ----------------
        with tc.For_i(0, NT, name="dec") as t:
            ntw = sb.tile([128, 2, 512], bf, tag="ntw")
            nc.sync.dma_start(out=ntw[:], in_=nodeT[:, :, ts(t, 512)])
            htb = sb.tile([128, 2, 512], bf, tag="htb")
            for m in range(2):
                hp = ps1.tile([128, 512], f32, tag="hp")
                for kc in range(2):
                    nc.tensor.matmul(hp[:], lhsT=wt['dec_w1'][:, kc, 128 * m:128 * m + 128],
                                     rhs=ntw[:, kc, :], start=(kc == 0), stop=(kc == 1))
                nc.scalar.activation(out=htb[:, m, :], in_=hp[:], func=AF.Silu,
                                     bias=wt['dec_b1'][:, m:m + 1])
            od = ps1.tile([128, 512], f32, tag="hp")
            for kc in range(2):
                nc.tensor.matmul(od[:], lhsT=wt['dec_w2'][:, kc, :],
                                 rhs=htb[:, kc, :], start=(kc == 0), stop=(kc == 1))
            odsb = sb.tile([128, 512], bf, tag="odsb")
            nc.scalar.activation(out=odsb[:], in_=od[:], func=AF.Identity,
                                 bias=wt['dec_b2'][:, 0:1])
            ow = sb.tile([128, 4, D_OUT], bf, tag="ow")
            for c in range(4):
                tp = ps2.tile([128, 128], bf, tag="sm")
                nc.tensor.transpose(out=tp[:], in_=odsb[:, 128 * c:128 * c + 128],
                                    identity=wt['ident'][:])
                if c % 2 == 0:
                    nc.vector.tensor_copy(out=ow[:, c, :], in_=tp[:])
                else:
                    nc.scalar.activation(out=ow[:, c, :], in_=tp[:], func=AF.Copy)
            nc.sync.dma_start(
                out=t_out.ap()[ts(t, 512)].rearrange("(c p) d -> p c d", p=128),
                in_=ow[:])

        if debug:
            for name, src in [("d_nodeT", nodeT), ("d_edgeT", edge_T),
                              ("d_aggT", aggT), ("d_C", C_d), ("d_CT", CT_d),
                              ("d_cc", cc_out), ("d_nloc", node_loc)]:
                dst = nc.dram_tensor(name, list(src.shape), bf,
                                     kind="ExternalOutput")
                tot = int(np.prod(src.shape))
                nchunk = -(-tot // (128 * 4096))
                while tot % nchunk or (tot // nchunk) % 128:
                    nchunk += 1
                step_e = tot // nchunk
                for i in range(nchunk):
                    tmp = sb.tile([128, step_e // 128], bf, tag="dbg")
                    nc.sync.dma_start(
                        out=tmp[:],
                        in_=src.ap().rearrange(
                            *(["a b c -> (a b c)"] if len(src.shape) == 3
                              else ["a b -> (a b)"]))[i * step_e:(i + 1) * step_e]
                        .rearrange("(p c) -> p c", p=128))
                    nc.sync.dma_start(
                        out=dst.ap().rearrange(
                            *(["a b c -> (a b c)"] if len(src.shape) == 3
                              else ["a b -> (a b)"]))[i * step_e:(i + 1) * step_e]
                        .rearrange("(p c) -> p c", p=128),
                        in_=tmp[:])
        stack.close()
    nc.finalize()
    return nc


# ----------------------------------------------------------------------------
# host wrapper
# ----------------------------------------------------------------------------

LAST_EXEC_NS = None
_PROG = None


def kernel(**inputs):
    global LAST_EXEC_NS, _PROG
    import os, time
    try:
        import jax
        jax.config.update('jax_compilation_cache_dir', '/tmp/gnn_jax_cache')
        jax.config.update('jax_persistent_cache_min_compile_time_secs', 0.0)
        jax.config.update('jax_persistent_cache_min_entry_size_bytes', -1)
    except Exception:
        pass
    inputs = {k: np.asarray(v) for k, v in inputs.items()}
    n_nodes = inputs['node_features'].shape[0]

    t0 = time.time()
    graph = _prep_graph(inputs['senders'], inputs['receivers'],
                        inputs['edge_features'])
    blob_bf, blob_f32, eew1 = _prep_weights(inputs)
    nf = np.asarray(inputs['node_features'], F32)
    in_maps = []
    c0_nf = BF_OFF['nfT'][0]
    c0_rc = F32_OFF['rcvc'][0]
    for k in range(NC):
        bb = blob_bf.copy()
        real = min(CHUNK_REAL, n_nodes - k * CHUNK_REAL)
        bb[:, c0_nf:c0_nf + real] = \
            nf[k * CHUNK_REAL:k * CHUNK_REAL + real].T.astype(BF16)
        bf32 = blob_f32.copy()
        bf32[:, c0_rc:c0_rc + ES // 128] = graph[k]['rcvc']
        in_maps.append(dict(blob_bf=bb, blob_f32=bf32, enc_e_w1=eew1,
                            efT=graph[k]['efT'], snd16=graph[k]['snd']))
    print(f"[kernel] host prep {time.time()-t0:.1f}s", flush=True)

    t0 = time.time()
    if _PROG is None:
        _PROG = build_program()
    print(f"[kernel] build {time.time()-t0:.1f}s", flush=True)

    from concourse.bass_utils import run_bass_kernel_spmd
    t0 = time.time()
    res = run_bass_kernel_spmd(_PROG, in_maps, core_ids=list(range(NC)))
    t1 = time.time()
    print(f"[kernel] run {t1-t0:.1f}s", flush=True)
    LAST_EXEC_NS = res.exec_time_ns
    if os.environ.get("GNN_TIME2"):
        t0 = time.time()
        res = run_bass_kernel_spmd(_PROG, in_maps, core_ids=list(range(NC)))
        t2 = time.time()
        print(f"[kernel] warm run {t2-t0:.1f}s", flush=True)
        LAST_EXEC_NS = int((t2 - t0) * 1e9)

    out = np.empty((n_nodes, D_OUT), np.float32)
    for k in range(NC):
        real = min(CHUNK_REAL, n_nodes - k * CHUNK_REAL)
        out[k * CHUNK_REAL:k * CHUNK_REAL + real] = \
            np.asarray(res.results[k]['outp'][:real], np.float32)
    return out


# revision 10
# speedup vs baseline: 1.0446x; 1.0446x over previous
"""DeepTypedGraphNet (GNN message passing) Trainium2 kernel v2, 8-core SPMD.

Design (vs v1 baseline): the whole program runs in hardware For_i loops with
affine addressing, cutting the BIR from ~100k to ~2k instructions (host build
+ rust compile + NEFF all shrink ~50x). Edges are packed per receiver-window:
core k owns nodes [12500k, 12500(k+1)), padded to 12800 = 100 windows of 128
nodes; each window gets 1024 edge slots = 8 chunks of 128, chunk j serving
sender-bank j//2 (4 banks of 25600 padded-global rows keep dma_gather's int16
index range). Receiver latents are expanded per chunk with a one-hot C matrix
(built on device from iota + is_equal); segment-sum aggregation is the same C
matrix used as matmul lhsT accumulating in PSUM -- no scatter, no agg tables.
Everything lives in latent-major (T) layout; LayerNorm reduces over latent =
partitions via ones-matmul column sums that broadcast for free. Sender latents
come from an AllGathered padded node table via dma_gather(transpose=True).
"""
import sys
sys.path.insert(0, '/opt/trn_rl_repo')

import numpy as np
import ml_dtypes

import concourse.bacc as bacc
import concourse.mybir as mybir
import concourse.tile as tile
from concourse.bass import ds, ts

BF16 = ml_dtypes.bfloat16
F32 = np.float32

LN_EPS = 1e-5
LATENT = 256
HIDDEN = 256
D_NODE_IN = 128
D_EDGE_IN = 4
D_OUT = 128
STEPS = 6

NC = 8
CHUNK_REAL = 12500
CHUNK = 12800            # 100 windows * 128
NW = CHUNK // 128        # 100 receiver windows per core
TAB = NC * CHUNK         # 102400 padded global nodes
NBANK = 4
BANK = TAB // NBANK      # 25600 (nominal bank stride; int16 reaches 32767)
QUOTA = 128              # slots per (window, bank) = 1 chunk
WSLOTS = NBANK * QUOTA   # 512 slots per window
ES = NW * WSLOTS         # 51200 edge slots per core
NT = CHUNK // 512        # 25 node pieces of 512

# blob layouts: (name, tile_shape); column count = prod(shape[1:])
BFSPEC = [
    ('nfT', (128, CHUNK)), ('ident', (128, 128)),
    ('enc_n_w1', (128, 1, 256)), ('enc_n_b1', (128, 2)),
    ('enc_n_w2', (128, 2, 256)), ('enc_n_b2', (128, 2)),
    ('enc_e_b1', (128, 2)), ('enc_e_w2', (128, 2, 256)), ('enc_e_b2', (128, 2)),
    ('pe_w1', (128, 6 * STEPS, 256)), ('pe_b1', (128, 2 * STEPS)),
    ('pe_w2', (128, 2 * STEPS, 256)), ('pe_b2', (128, 2 * STEPS)),
    ('pn_w1', (128, 4 * STEPS, 256)), ('pn_b1', (128, 2 * STEPS)),
    ('pn_w2', (128, 2 * STEPS, 256)), ('pn_b2', (128, 2 * STEPS)),
    ('dec_w1', (128, 2, 256)), ('dec_b1', (128, 2)),
    ('dec_w2', (128, 2, 128)), ('dec_b2', (128, 1)),
]
F32SPEC = [
    ('iota', (128, 128)), ('rcvc', (128, ES // 128)),
    ('enc_n_s', (128, 2)), ('enc_n_o', (128, 2)),
    ('enc_e_s', (128, 2)), ('enc_e_o', (128, 2)),
    ('pe_s', (128, 2 * STEPS)), ('pe_o', (128, 2 * STEPS)),
    ('pn_s', (128, 2 * STEPS)), ('pn_o', (128, 2 * STEPS)),
]


def _blob_offsets(spec):
    off = {}
    c = 0
    for name, shape in spec:
        n = int(np.prod(shape[1:]))
        off[name] = (c, n, shape)
        c += n
    return off, c


BF_OFF, BF_COLS = _blob_offsets(BFSPEC)
F32_OFF, F32_COLS = _blob_offsets(F32SPEC)


# ----------------------------------------------------------------------------
# host-side prep
# ----------------------------------------------------------------------------

def _wrap_idx(vals):
    """[n] int16 -> [16, n/16] wrapped: slot i at [i%16, i//16]. The required
    replication across the 8 groups of 16 partitions happens on device."""
    n = len(vals)
    a = np.asarray(vals, np.int16).reshape(n // 16, 16).T
    return np.ascontiguousarray(a)


def _prep_graph(senders, receivers, edge_features):
    s = np.asarray(senders, np.int64)
    r = np.asarray(receivers, np.int64)
    ef = np.asarray(edge_features, F32)
    E = len(s)
    ks = s // CHUNK_REAL
    ps = ks * CHUNK + (s - ks * CHUNK_REAL)      # padded global sender id
    kr = r // CHUNK_REAL
    rl = r - kr * CHUNK_REAL                     # local receiver id
    w = rl // 128
    pos = rl % 128
    b = ps // BANK

    # QUOTA=128: a few (core,window,bank) cells exceed 128 edges. int16 idx
    # reaches 32767 > BANK, so an edge with ps - (b-1)*BANK <= 32767 can be
    # served from the lower bank's gather base. Move overflow edges down.
    cell = ((kr * NW + w) * NBANK + b)
    cnt = np.bincount(cell, minlength=NC * NW * NBANK)
    for c in np.nonzero(cnt > QUOTA)[0][::-1]:  # higher banks first
        j = c % NBANK
        if j == 0:
            raise RuntimeError("bank-0 overflow: cannot borrow downward")
        over = cnt[c] - QUOTA
        cand = np.nonzero((cell == c) & (ps - (j - 1) * BANK <= 32767))[0]
        if len(cand) < over or cnt[c - 1] + over > QUOTA:
            raise RuntimeError("overflow not resolvable by bank borrowing")
        b[cand[:over]] -= 1
        cnt[c] -= over
        cnt[c - 1] += over
    group = ((kr * NW + w) * NBANK + b)
    order = np.argsort(group, kind='stable')
    g_sorted = group[order]
    # index within group
    uniq, starts, counts = np.unique(g_sorted, return_index=True, return_counts=True)
    if counts.max() > QUOTA:
        raise RuntimeError(f"(window,bank) overflow: {counts.max()} > {QUOTA}")
    within = np.arange(E) - np.repeat(starts, counts)
    slot = g_sorted * QUOTA + within              # global slot id: core*ES + local
    core = g_sorted // (NW * NBANK)
    local_slot = slot - core * (NW * NBANK * QUOTA)

    out = []
    for k in range(NC):
        sel = core == k
        e_ids = order[sel]
        sl = local_slot[sel]
        snd_rel = np.zeros(ES, np.int16)
        rcv_val = np.full(ES, 255.0, F32)
        efT = np.zeros((D_EDGE_IN, ES), F32)
        snd_rel[sl] = (ps[e_ids] - b[e_ids] * BANK).astype(np.int16)
        rcv_val[sl] = pos[e_ids].astype(F32)
        efT[:, sl] = ef[e_ids].T
        rcv_col = np.ascontiguousarray(rcv_val.reshape(ES // 128, 128).T)  # [128, 800]
        eid = np.full(ES, -1, np.int64)
        eid[sl] = e_ids
        out.append(dict(snd=_wrap_idx(snd_rel), rcvc=rcv_col,
                        efT=efT.astype(BF16), _eid=eid))
    return out


def _prep_weights(i):
    """Pack weights into the kernel layouts (bf16 unless noted)."""
    w = {}

    def w1pack(mat):  # [K, N] -> [128, K/128, N] lhsT chunks
        K, N = mat.shape
        assert K % 128 == 0
        return np.ascontiguousarray(
            np.asarray(mat, F32).reshape(K // 128, 128, N).transpose(1, 0, 2))

    def cols(vec_s):  # [S, 256] -> [128, S*2]
        v = np.asarray(vec_s, F32)
        if v.ndim == 1:
            v = v[None]
        S = v.shape[0]
        return np.ascontiguousarray(v.reshape(S, 2, 128).transpose(2, 0, 1).reshape(128, S * 2))

    w['enc_n_w1'] = w1pack(i['enc_node_w1'])                       # [128,1,256]
    w['enc_n_b1'] = cols(i['enc_node_b1'])
    w['enc_n_w2'] = w1pack(i['enc_node_w2'])                       # [128,2,256]
    w['enc_n_b2'] = cols(i['enc_node_b2'])
    w['enc_n_s'] = cols(i['enc_node_ln_s'])
    w['enc_n_o'] = cols(i['enc_node_ln_o'])
    w['enc_e_w1'] = np.asarray(i['enc_edge_w1'], F32)[:, None, :]  # [4,1,256]
    w['enc_e_b1'] = cols(i['enc_edge_b1'])
    w['enc_e_w2'] = w1pack(i['enc_edge_w2'])
    w['enc_e_b2'] = cols(i['enc_edge_b2'])
    w['enc_e_s'] = cols(i['enc_edge_ln_s'])
    w['enc_e_o'] = cols(i['enc_edge_ln_o'])

    w['pe_w1'] = np.concatenate([w1pack(i['pe_w1'][s]) for s in range(STEPS)], 1)  # [128,36,256]
    w['pe_b1'] = cols(i['pe_b1'])
    w['pe_w2'] = np.concatenate([w1pack(i['pe_w2'][s]) for s in range(STEPS)], 1)  # [128,12,256]
    w['pe_b2'] = cols(i['pe_b2'])
    w['pe_s'] = cols(i['pe_ln_s'])
    w['pe_o'] = cols(i['pe_ln_o'])
    w['pn_w1'] = np.concatenate([w1pack(i['pn_w1'][s]) for s in range(STEPS)], 1)  # [128,24,256]
    w['pn_b1'] = cols(i['pn_b1'])
    w['pn_w2'] = np.concatenate([w1pack(i['pn_w2'][s]) for s in range(STEPS)], 1)
    w['pn_b2'] = cols(i['pn_b2'])
    w['pn_s'] = cols(i['pn_ln_s'])
    w['pn_o'] = cols(i['pn_ln_o'])

    w['dec_w1'] = w1pack(i['dec_w1'])                              # [128,2,256]
    w['dec_b1'] = cols(i['dec_b1'])
    w['dec_w2'] = w1pack(np.asarray(i['dec_w2'], F32))             # [128,2,128]
    w['dec_b2'] = np.asarray(i['dec_b2'], F32)[:, None]            # [128,1]

    w['iota'] = np.tile(np.arange(128, dtype=F32)[None, :], (128, 1))
    w['ident'] = np.eye(128, dtype=F32)

    # pack into blobs (nfT / rcvc regions are per-core, filled by caller)
    blob_bf = np.zeros((128, BF_COLS), BF16)
    for name, (c0, n, shape) in BF_OFF.items():
        if name == 'nfT':
            continue
        blob_bf[:, c0:c0 + n] = w[name].reshape(128, n).astype(BF16)
    blob_f32 = np.zeros((128, F32_COLS), F32)
    for name, (c0, n, shape) in F32_OFF.items():
        if name == 'rcvc':
            continue
        blob_f32[:, c0:c0 + n] = w[name].reshape(128, n).astype(F32)
    return blob_bf, blob_f32, np.ascontiguousarray(w['enc_e_w1'].astype(BF16))


# ----------------------------------------------------------------------------
# program
# ----------------------------------------------------------------------------

def build_program(steps=STEPS, debug=False):
    dt = mybir.dt
    bf = dt.bfloat16
    f32 = dt.float32
    AF = mybir.ActivationFunctionType
    OP = mybir.AluOpType

    nc = bacc.Bacc(None, target_bir_lowering=False)

    def inp(name, shape, dtype=bf):
        return nc.dram_tensor(name, shape, dtype, kind="ExternalInput")

    t_bf = inp("blob_bf", [128, BF_COLS])
    t_f32 = inp("blob_f32", [128, F32_COLS], f32)
    t_efT = inp("efT", [D_EDGE_IN, ES])
    t_snd = inp("snd16", [16, ES // 16], dt.int16)
    t_eew1 = inp("enc_e_w1", [D_EDGE_IN, 1, 256])

    t_out = nc.dram_tensor("outp", [CHUNK, D_OUT], bf, kind="ExternalOutput")

    # internal DRAM
    node_loc = nc.dram_tensor("node_loc", [CHUNK, LATENT], bf)
    cc_out = nc.dram_tensor("cc_out", [TAB, LATENT], bf, addr_space="Shared")
    edge_T = nc.dram_tensor("edge_T", [128, 2, ES], bf)
    nodeT = nc.dram_tensor("nodeT", [128, 2, CHUNK], bf)
    aggT = nc.dram_tensor("aggT", [128, 2, CHUNK], bf)
    C_d = nc.dram_tensor("C_d", [128, ES], bf)
    CT_d = nc.dram_tensor("CT_d", [128, ES], bf)
    snd_x = nc.dram_tensor("snd_x", [128, ES // 16], dt.int16)

    with tile.TileContext(nc) as tc:
        import contextlib
        stack = contextlib.ExitStack()
        wp = stack.enter_context(tc.tile_pool(name="wp", bufs=1))
        sb = stack.enter_context(tc.tile_pool(name="sb", bufs=2))
        ps2 = stack.enter_context(tc.tile_pool(name="ps2", bufs=2, space="PSUM"))
        ps1 = stack.enter_context(tc.tile_pool(name="ps1", bufs=1, space="PSUM"))

        wt = {}
        for name, (c0, ncols, shape) in list(BF_OFF.items()) + list(F32_OFF.items()):
            if name in ('nfT', 'rcvc'):
                continue
            if name in dict(F32SPEC):
                blob, dtype = t_f32, f32
            else:
                blob, dtype = t_bf, bf
            t = wp.tile(list(shape), dtype, tag=f"w_{name}")
            src = blob.ap()[:, c0:c0 + ncols]
            if len(shape) == 3:
                src = src.rearrange("p (k n) -> p k n", k=shape[1])
            nc.sync.dma_start(out=t[:], in_=src)
            wt[name] = t
        t = wp.tile([D_EDGE_IN, 1, 256], bf, tag="w_enc_e_w1")
        nc.sync.dma_start(out=t[:], in_=t_eew1[:])
        wt['enc_e_w1'] = t
        ones_t = wp.tile([128, 128], bf, tag="ones")
        nc.vector.memset(ones_t[:], 1.0)
        eps_t = wp.tile([128, 1], f32, tag="eps")
        nc.vector.memset(eps_t[:], LN_EPS)

        # expand snd16 [16, ES/16] -> snd_x [128, ES/16] (8 Q7-core replicas)
        s16 = wp.tile([16, ES // 16], dt.int16, tag="s16")
        nc.sync.dma_start(out=s16[:], in_=t_snd[:])
        for g in range(8):
            nc.sync.dma_start(out=snd_x.ap()[16 * g:16 * g + 16, :], in_=s16[:])

        def ln_half(o2, kb2, ks, ko, resid_ap, out_ap, fw):
            """PSUM o2 [128,2,FW] f32 -> LayerNorm over latent(=partitions,
            2 chunks) with scale/offset cols, optional residual, bf16 out."""
            o2sb = sb.tile([128, 2, 512], f32, tag="o2sb")
            o2bf = sb.tile([128, 2, 512], bf, tag="o2bf")
            sq = sb.tile([128, 2, 512], bf, tag="sq")
            for lc in range(2):
                nc.scalar.activation(out=o2sb[:, lc, :fw], in_=o2[:, lc, :fw],
                                     func=AF.Identity, bias=kb2[:, lc:lc + 1])
                nc.vector.tensor_copy(out=o2bf[:, lc, :fw], in_=o2sb[:, lc, :fw])
                nc.scalar.activation(out=sq[:, lc, :fw], in_=o2sb[:, lc, :fw],
                                     func=AF.Square)
            S = ps1.tile([128, 512], f32, tag="S")
            nc.tensor.matmul(S[:, :fw], lhsT=ones_t[:], rhs=o2bf[:, 0, :fw],
                             start=True, stop=False)
            nc.tensor.matmul(S[:, :fw], lhsT=ones_t[:], rhs=o2bf[:, 1, :fw],
                             start=False, stop=True)
            S2 = ps1.tile([128, 512], f32, tag="S2")
            nc.tensor.matmul(S2[:, :fw], lhsT=ones_t[:], rhs=sq[:, 0, :fw],
                             start=True, stop=False)
            nc.tensor.matmul(S2[:, :fw], lhsT=ones_t[:], rhs=sq[:, 1, :fw],
                             start=False, stop=True)
            m = sb.tile([128, 512], f32, tag="st_m")
            v = sb.tile([128, 512], f32, tag="st_v")
            nc.vector.tensor_scalar(out=m[:, :fw], in0=S[:, :fw], scalar1=1.0 / LATENT,
                                    scalar2=None, op0=OP.mult)
            nc.vector.tensor_scalar(out=v[:, :fw], in0=S2[:, :fw], scalar1=1.0 / LATENT,
                                    scalar2=None, op0=OP.mult)
            msq = sb.tile([128, 512], f32, tag="st_m2")
            nc.vector.tensor_tensor(out=msq[:, :fw], in0=m[:, :fw], in1=m[:, :fw],
                                    op=OP.mult)
            nc.vector.tensor_tensor(out=v[:, :fw], in0=v[:, :fw], in1=msq[:, :fw],
                                    op=OP.subtract)
            inv = sb.tile([128, 512], f32, tag="st_i")
            nc.scalar.activation(out=inv[:, :fw], in_=v[:, :fw], func=AF.Sqrt,
                                 bias=eps_t[:])
            nc.vector.reciprocal(inv[:, :fw], inv[:, :fw])
            b = sb.tile([128, 512], f32, tag="st_b")
            nc.vector.tensor_tensor(out=b[:, :fw], in0=m[:, :fw], in1=inv[:, :fw],
                                    op=OP.mult)
            for lc in range(2):
                y = sb.tile([128, 512], f32, tag="st_y")
                nc.vector.tensor_tensor(out=y[:, :fw], in0=o2sb[:, lc, :fw],
                                        in1=inv[:, :fw], op=OP.mult)
                nc.vector.tensor_tensor(out=y[:, :fw], in0=y[:, :fw], in1=b[:, :fw],
                                        op=OP.subtract)
                z = sb.tile([128, 512], f32, tag="st_z")
                nc.vector.tensor_scalar(out=z[:, :fw], in0=y[:, :fw],
                                        scalar1=ks[:, lc:lc + 1],
                                        scalar2=ko[:, lc:lc + 1],
                                        op0=OP.mult, op1=OP.add)
                if resid_ap is not None:
                    nc.vector.tensor_tensor(out=out_ap[lc], in0=z[:, :fw],
                                            in1=resid_ap[lc], op=OP.add)
                else:
                    nc.vector.tensor_copy(out=out_ap[lc], in_=z[:, :fw])

        # ---------------- C build ----------------
        NCH = WSLOTS // 128  # chunks per window
        with tc.For_i(0, NW, name="cbuild") as w:
            rc = sb.tile([128, NCH], f32, tag="rc")
            nc.sync.dma_start(out=rc[:], in_=t_f32[:, ds(w * NCH + F32_OFF['rcvc'][0], NCH)])
            cw = sb.tile([128, WSLOTS], bf, tag="cw")
            ctw = sb.tile([128, WSLOTS], bf, tag="ctw")
            for j in range(NCH):
                nc.vector.tensor_scalar(out=cw[:, 128 * j:128 * j + 128],
                                        in0=wt['iota'][:], scalar1=rc[:, j:j + 1],
                                        scalar2=None, op0=OP.is_equal)
                tp = ps2.tile([128, 128], bf, tag="sm")
                nc.tensor.transpose(out=tp[:], in_=cw[:, 128 * j:128 * j + 128],
                                    identity=wt['ident'][:])
                if j % 2 == 0:
                    nc.vector.tensor_copy(out=ctw[:, 128 * j:128 * j + 128], in_=tp[:])
                else:
                    nc.scalar.activation(out=ctw[:, 128 * j:128 * j + 128], in_=tp[:],
                                         func=AF.Copy)
            nc.sync.dma_start(out=C_d[:, ts(w, WSLOTS)], in_=cw[:])
            nc.sync.dma_start(out=CT_d[:, ts(w, WSLOTS)], in_=ctw[:])

        # ---------------- node encoder ----------------
        def mlp_tail(htb, wname, bname, sname, oname, sidx, resid, outf, t, dram,
                     fw=512):
            """L2 (+bias) + LN + optional residual + store to dram[:, :, ts]."""
            o2 = ps1.tile([128, 2, 512], f32, tag="o2")
            for lc in range(2):
                for kc in range(2):
                    nc.tensor.matmul(o2[:, lc, :fw],
                                     lhsT=wt[wname][:, 2 * sidx + kc, 128 * lc:128 * lc + 128],
                                     rhs=htb[:, kc, :fw],
                                     start=(kc == 0), stop=(kc == 1))
            newt = sb.tile([128, 2, 512], bf, tag="newt")
            resid_ap = None
            if resid is not None:
                resid_ap = [resid[:, lc, :fw] for lc in range(2)]
            ln_half(o2, wt[bname][:, 2 * sidx:2 * sidx + 2],
                    wt[sname][:, 2 * sidx:2 * sidx + 2],
                    wt[oname][:, 2 * sidx:2 * sidx + 2],
                    resid_ap, [newt[:, lc, :fw] for lc in range(2)], fw)
            nc.sync.dma_start(out=dram[:, :, ts(t, fw)], in_=newt[:, :, :fw])
            return newt

        def store_normal(newt, t, dram, nch=4, dtile=LATENT):
            """Transpose [128,2,512]T tile -> normal rows, write dram rows."""
            nn = sb.tile([128, 4, 256], dram.dtype, tag="nn4")
            for c in range(nch):
                for lc in range(2):
                    tp = ps2.tile([128, 128], bf, tag="sm")
                    nc.tensor.transpose(out=tp[:], in_=newt[:, lc, 128 * c:128 * c + 128],
                                        identity=wt['ident'][:])
                    if (c + lc) % 2 == 0:
                        nc.vector.tensor_copy(out=nn[:, c, 128 * lc:128 * lc + 128], in_=tp[:])
                    else:
                        nc.scalar.activation(out=nn[:, c, 128 * lc:128 * lc + 128],
                                             in_=tp[:], func=AF.Copy)
            nc.sync.dma_start(
                out=dram.ap()[ts(t, 512)].rearrange("(c p) d -> p c d", p=128),
                in_=nn[:, :nch, :])

        with tc.For_i(0, NT, name="enc_n") as t:
            nf = sb.tile([128, 512], bf, tag="nf")
            nc.sync.dma_start(out=nf[:], in_=t_bf[:, ds(t * 512 + BF_OFF['nfT'][0], 512)])
            htb = sb.tile([128, 2, 512], bf, tag="htb")
            for m in range(2):
                hp = ps1.tile([128, 512], f32, tag="hp")
                nc.tensor.matmul(hp[:], lhsT=wt['enc_n_w1'][:, 0, 128 * m:128 * m + 128],
                                 rhs=nf[:], start=True, stop=True)
                nc.scalar.activation(out=htb[:, m, :], in_=hp[:], func=AF.Silu,
                                     bias=wt['enc_n_b1'][:, m:m + 1])
            newt = mlp_tail(htb, 'enc_n_w2', 'enc_n_b2', 'enc_n_s', 'enc_n_o', 0,
                            None, None, t, nodeT)
            store_normal(newt, t, node_loc)

        # ---------------- edge encoder ----------------
        with tc.For_i(0, NW, name="enc_e") as w:
            ef = sb.tile([D_EDGE_IN, WSLOTS], bf, tag="ef")
            nc.sync.dma_start(out=ef[:], in_=t_efT[:, ts(w, WSLOTS)])
            htb = sb.tile([128, 2, 512], bf, tag="htb")
            for m in range(2):
                hp = ps1.tile([128, 512], f32, tag="hp")
                nc.tensor.matmul(hp[:], lhsT=wt['enc_e_w1'][:, 0, 128 * m:128 * m + 128],
                                 rhs=ef[:], start=True, stop=True)
                nc.scalar.activation(out=htb[:, m, :], in_=hp[:], func=AF.Silu,
                                     bias=wt['enc_e_b1'][:, m:m + 1])
            o2 = ps1.tile([128, 2, 512], f32, tag="o2")
            for lc in range(2):
                for kc in range(2):
                    nc.tensor.matmul(o2[:, lc, :],
                                     lhsT=wt['enc_e_w2'][:, kc, 128 * lc:128 * lc + 128],
                                     rhs=htb[:, kc, :], start=(kc == 0), stop=(kc == 1))
            newt = sb.tile([128, 2, 512], bf, tag="newt")
            ln_half(o2, wt['enc_e_b2'][:, 0:2], wt['enc_e_s'][:, 0:2],
                    wt['enc_e_o'][:, 0:2], None,
                    [newt[:, lc, :] for lc in range(2)], 512)
            nc.sync.dma_start(out=edge_T[:, :, ts(w, WSLOTS)], in_=newt[:])

        # ---------------- message passing ----------------
        for s in range(steps):
            nc.gpsimd.collective_compute(
                "AllGather", mybir.AluOpType.bypass,
                ins=[node_loc[:]], outs=[cc_out[:]],
                replica_groups=[list(range(NC))])

            with tc.For_i(0, NW, name=f"edge{s}") as w:
                xe = sb.tile([128, 2, WSLOTS], bf, tag="xe")
                nc.sync.dma_start(out=xe[:], in_=edge_T[:, :, ts(w, WSLOTS)])
                idxt = sb.tile([128, WSLOTS // 16], mybir.dt.int16, tag="idxt")
                nc.sync.dma_start(out=idxt[:], in_=snd_x.ap()[:, ts(w, WSLOTS // 16)])
                xst = sb.tile([128, NBANK, 2, QUOTA], bf, tag="xst")
                for b in range(NBANK):
                    nc.gpsimd.dma_gather(
                        out_ap=xst[:, b, :, :],
                        in_ap=cc_out.ap()[BANK * b:min(BANK * b + 32768, TAB)],
                        idxs_ap=idxt[:, (QUOTA // 16) * b:(QUOTA // 16) * (b + 1)],
                        num_idxs=QUOTA, num_idxs_reg=QUOTA, elem_size=LATENT,
                        transpose=True)
                xs = sb.tile([128, 2, WSLOTS], bf, tag="xs")
                for b in range(NBANK):
                    for lc in range(2):
                        if (b + lc) % 2 == 0:
                            nc.vector.tensor_copy(out=xs[:, lc, QUOTA * b:QUOTA * b + QUOTA],
                                                  in_=xst[:, b, lc, :])
                        else:
                            nc.scalar.activation(out=xs[:, lc, QUOTA * b:QUOTA * b + QUOTA],
                                                 in_=xst[:, b, lc, :], func=AF.Copy)
                ndw = sb.tile([128, LATENT], bf, tag="ndw")
                nc.sync.dma_start(out=ndw[:], in_=node_loc.ap()[ts(w, 128)])
                ct = sb.tile([128, WSLOTS], bf, tag="ct")
                nc.sync.dma_start(out=ct[:], in_=CT_d[:, ts(w, WSLOTS)])
                cc = sb.tile([128, WSLOTS], bf, tag="cc")
                nc.sync.dma_start(out=cc[:], in_=C_d[:, ts(w, WSLOTS)])
                xr = sb.tile([128, 2, WSLOTS], bf, tag="xr")
                for j in range(NCH):
                    for lc in range(2):
                        xp = ps2.tile([128, 128], f32, tag="sm")
                        nc.tensor.matmul(xp[:], lhsT=ndw[:, 128 * lc:128 * lc + 128],
                                         rhs=ct[:, 128 * j:128 * j + 128],
                                         start=True, stop=True)
                        if (j + lc) % 2 == 0:
                            nc.vector.tensor_copy(out=xr[:, lc, 128 * j:128 * j + 128], in_=xp[:])
                        else:
                            nc.scalar.activation(out=xr[:, lc, 128 * j:128 * j + 128],
                                                 in_=xp[:], func=AF.Copy)

                newe = sb.tile([128, 2, WSLOTS], bf, tag="newe")
                htb = sb.tile([128, 2, 512], bf, tag="htb")
                for m in range(2):
                    hp = ps1.tile([128, 512], f32, tag="hp")
                    first = True
                    for src, kbase in ((xe, 0), (xs, 2), (xr, 4)):
                        for kc in range(2):
                            ci = kbase + kc
                            nc.tensor.matmul(
                                hp[:], lhsT=wt['pe_w1'][:, 6 * s + ci, 128 * m:128 * m + 128],
                                rhs=src[:, kc, :], start=first, stop=(ci == 5))
                            first = False
                    nc.scalar.activation(out=htb[:, m, :], in_=hp[:], func=AF.Silu,
                                         bias=wt['pe_b1'][:, 2 * s + m:2 * s + m + 1])
                o2 = ps1.tile([128, 2, 512], f32, tag="o2")
                for lc in range(2):
                    for kc in range(2):
                        nc.tensor.matmul(o2[:, lc, :],
                                         lhsT=wt['pe_w2'][:, 2 * s + kc, 128 * lc:128 * lc + 128],
                                         rhs=htb[:, kc, :], start=(kc == 0), stop=(kc == 1))
                ln_half(o2, wt['pe_b2'][:, 2 * s:2 * s + 2],
                        wt['pe_s'][:, 2 * s:2 * s + 2], wt['pe_o'][:, 2 * s:2 * s + 2],
                        [xe[:, lc, :] for lc in range(2)],
                        [newe[:, lc, :] for lc in range(2)], 512)
                nc.sync.dma_start(out=edge_T[:, :, ts(w, WSLOTS)], in_=newe[:])

                # aggregation: aggT[:, lc, w*128:] = sum_j newe_j^T @ C_j
                # (the two lc chains share one PSUM bank: keep each chain
                # contiguous -- a group's first matmul clears the whole
                # bank's has_written bits)
                agg = ps1.tile([128, 2, 128], f32, tag="agg")
                nn8 = sb.tile([128, NCH, 256], bf, tag="nn8")
                for j in range(NCH):
                    for lc in range(2):
                        tp = ps2.tile([128, 128], bf, tag="sm")
                        nc.tensor.transpose(out=tp[:], in_=newe[:, lc, 128 * j:128 * j + 128],
                                            identity=wt['ident'][:])
                        if lc == 0:
                            nc.vector.tensor_copy(out=nn8[:, j, 128 * lc:128 * lc + 128], in_=tp[:])
                        else:
                            nc.scalar.activation(out=nn8[:, j, 128 * lc:128 * lc + 128],
                                                 in_=tp[:], func=AF.Copy)
                for lc in range(2):
                    for j in range(NCH):
                        nc.tensor.matmul(agg[:, lc, :], lhsT=nn8[:, j, 128 * lc:128 * lc + 128],
                                         rhs=cc[:, 128 * j:128 * j + 128],
                                         start=(j == 0), stop=(j == NCH - 1))
                agsb = sb.tile([128, 2, 128], bf, tag="agsb")
                nc.vector.tensor_copy(out=agsb[:, 0, :], in_=agg[:, 0, :])
                nc.scalar.activation(out=agsb[:, 1, :], in_=agg[:, 1, :], func=AF.Copy)
                nc.sync.dma_start(out=aggT[:, :, ts(w, 128)], in_=agsb[:])

            with tc.For_i(0, NT, name=f"node{s}") as t:
                ntw = sb.tile([128, 2, 512], bf, tag="ntw")
                nc.sync.dma_start(out=ntw[:], in_=nodeT[:, :, ts(t, 512)])
                agw = sb.tile([128, 2, 512], bf, tag="agw")
                nc.sync.dma_start(out=agw[:], in_=aggT[:, :, ts(t, 512)])
                htb = sb.tile([128, 2, 512], bf, tag="htb")
                for m in range(2):
                    hp = ps1.tile([128, 512], f32, tag="hp")
                    first = True
                    for src, kbase in ((ntw, 0), (agw, 2)):
                        for kc in range(2):
                            ci = kbase + kc
                            nc.tensor.matmul(
                                hp[:], lhsT=wt['pn_w1'][:, 4 * s + ci, 128 * m:128 * m + 128],
                                rhs=src[:, kc, :], start=first, stop=(ci == 3))
                            first = False
                    nc.scalar.activation(out=htb[:, m, :], in_=hp[:], func=AF.Silu,
                                         bias=wt['pn_b1'][:, 2 * s + m:2 * s + m + 1])
                newt = mlp_tail(htb, 'pn_w2', 'pn_b2', 'pn_s', 'pn_o', s, ntw,
                                None, t, nodeT)
                store_normal(newt, t, node_loc)

        # ---------------- decoder ----------------
        with tc.For_i(0, NT, name="dec") as t:
            ntw = sb.tile([128, 2, 512], bf, tag="ntw")
            nc.sync.dma_start(out=ntw[:], in_=nodeT[:, :, ts(t, 512)])
            htb = sb.tile([128, 2, 512], bf, tag="htb")
            for m in range(2):
                hp = ps1.tile([128, 512], f32, tag="hp")
                for kc in range(2):
                    nc.tensor.matmul(hp[:], lhsT=wt['dec_w1'][:, kc, 128 * m:128 * m + 128],
                                     rhs=ntw[:, kc, :], start=(kc == 0), stop=(kc == 1))
                nc.scalar.activation(out=htb[:, m, :], in_=hp[:], func=AF.Silu,
                                     bias=wt['dec_b1'][:, m:m + 1])
            od = ps1.tile([128, 512], f32, tag="hp")
            for kc in range(2):
                nc.tensor.matmul(od[:], lhsT=wt['dec_w2'][:, kc, :],
                                 rhs=htb[:, kc, :], start=(kc == 0), stop=(kc == 1))
            odsb = sb.tile([128, 512], bf, tag="odsb")
            nc.scalar.activation(out=odsb[:], in_=od[:], func=AF.Identity,
                                 bias=wt['dec_b2'][:, 0:1])
            ow = sb.tile([128, 4, D_OUT], bf, tag="ow")
            for c in range(4):
                tp = ps2.tile([128, 128], bf, tag="sm")
                nc.tensor.transpose(out=tp[:], in_=odsb[:, 128 * c:128 * c + 128],
                                    identity=wt['ident'][:])
                if c % 2 == 0:
                    nc.vector.tensor_copy(out=ow[:, c, :], in_=tp[:])
                else:
                    nc.scalar.activation(out=ow[:, c, :], in_=tp[:], func=AF.Copy)
            nc.sync.dma_start(
                out=t_out.ap()[ts(t, 512)].rearrange("(c p) d -> p c d", p=128),
                in_=ow[:])

        if debug:
            for name, src in [("d_nodeT", nodeT), ("d_edgeT", edge_T),
                              ("d_aggT", aggT), ("d_C", C_d), ("d_CT", CT_d),
                              ("d_cc", cc_out), ("d_nloc", node_loc)]:
                dst = nc.dram_tensor(name, list(src.shape), bf,
                                     kind="ExternalOutput")
                tot = int(np.prod(src.shape))
                nchunk = -(-tot // (128 * 4096))
                while tot % nchunk or (tot // nchunk) % 128:
                    nchunk += 1
                step_e = tot // nchunk
                for i in range(nchunk):
                    tmp = sb.tile([128, step_e // 128], bf, tag="dbg")
                    nc.sync.dma_start(
                        out=tmp[:],
                        in_=src.ap().rearrange(
                            *(["a b c -> (a b c)"] if len(src.shape) == 3
                              else ["a b -> (a b)"]))[i * step_e:(i + 1) * step_e]
                        .rearrange("(p c) -> p c", p=128))
                    nc.sync.dma_start(
                        out=dst.ap().rearrange(
                            *(["a b c -> (a b c)"] if len(src.shape) == 3
                              else ["a b -> (a b)"]))[i * step_e:(i + 1) * step_e]
                        .rearrange("(p c) -> p c", p=128),
                        in_=tmp[:])
        stack.close()
    nc.finalize()
    return nc


# ----------------------------------------------------------------------------
# host wrapper
# ----------------------------------------------------------------------------

LAST_EXEC_NS = None
_PROG = None


def kernel(**inputs):
    global LAST_EXEC_NS, _PROG
    import os, time
    try:
        import jax
        jax.config.update('jax_compilation_cache_dir', '/tmp/gnn_jax_cache')
        jax.config.update('jax_persistent_cache_min_compile_time_secs', 0.0)
        jax.config.update('jax_persistent_cache_min_entry_size_bytes', -1)
    except Exception:
        pass
    inputs = {k: np.asarray(v) for k, v in inputs.items()}
    n_nodes = inputs['node_features'].shape[0]

    t0 = time.time()
    graph = _prep_graph(inputs['senders'], inputs['receivers'],
                        inputs['edge_features'])
    blob_bf, blob_f32, eew1 = _prep_weights(inputs)
    nf = np.asarray(inputs['node_features'], F32)
    in_maps = []
    c0_nf = BF_OFF['nfT'][0]
    c0_rc = F32_OFF['rcvc'][0]
    for k in range(NC):
        bb = blob_bf.copy()
        real = min(CHUNK_REAL, n_nodes - k * CHUNK_REAL)
        bb[:, c0_nf:c0_nf + real] = \
            nf[k * CHUNK_REAL:k * CHUNK_REAL + real].T.astype(BF16)
        bf32 = blob_f32.copy()
        bf32[:, c0_rc:c0_rc + ES // 128] = graph[k]['rcvc']
        in_maps.append(dict(blob_bf=bb, blob_f32=bf32, enc_e_w1=eew1,
                            efT=graph[k]['efT'], snd16=graph[k]['snd']))
    print(f"[kernel] host prep {time.time()-t0:.1f}s", flush=True)

    t0 = time.time()
    if _PROG is None:
        _PROG = build_program()
    print(f"[kernel] build {time.time()-t0:.1f}s", flush=True)

    from concourse.bass_utils import run_bass_kernel_spmd
    t0 = time.time()
    res = run_bass_kernel_spmd(_PROG, in_maps, core_ids=list(range(NC)))
    t1 = time.time()
    print(f"[kernel] run {t1-t0:.1f}s", flush=True)
    LAST_EXEC_NS = res.exec_time_ns
    if os.environ.get("GNN_TIME2"):
        t0 = time.time()
        res = run_bass_kernel_spmd(_PROG, in_maps, core_ids=list(range(NC)))
        t2 = time.time()
        print(f"[kernel] warm run {t2-t0:.1f}s", flush=True)
        LAST_EXEC_NS = int((t2 - t0) * 1e9)

    out = np.empty((n_nodes, D_OUT), np.float32)
    for k in range(NC):
        real = min(CHUNK_REAL, n_nodes - k * CHUNK_REAL)
        out[k * CHUNK_REAL:k * CHUNK_REAL + real] = \
            np.asarray(res.results[k]['outp'][:real], np.float32)
    return out


# revision 18
# speedup vs baseline: 1.4894x; 1.4258x over previous
"""DeepTypedGraphNet (GNN message passing) Trainium2 kernel v2, 8-core SPMD.

Design (vs v1 baseline): the whole program runs in hardware For_i loops with
affine addressing, cutting the BIR from ~100k to ~2k instructions (host build
+ rust compile + NEFF all shrink ~50x). Edges are packed per receiver-window:
core k owns nodes [12500k, 12500(k+1)), padded to 12800 = 100 windows of 128
nodes; each window gets 1024 edge slots = 8 chunks of 128, chunk j serving
sender-bank j//2 (4 banks of 25600 padded-global rows keep dma_gather's int16
index range). Receiver latents are expanded per chunk with a one-hot C matrix
(built on device from iota + is_equal); segment-sum aggregation is the same C
matrix used as matmul lhsT accumulating in PSUM -- no scatter, no agg tables.
Everything lives in latent-major (T) layout; LayerNorm reduces over latent =
partitions via ones-matmul column sums that broadcast for free. Sender latents
come from an AllGathered padded node table via dma_gather(transpose=True).
"""
import sys
sys.path.insert(0, '/opt/trn_rl_repo')

import numpy as np
import ml_dtypes

import concourse.bacc as bacc
import concourse.mybir as mybir
import concourse.tile as tile
from concourse.bass import ds, ts

BF16 = ml_dtypes.bfloat16
F32 = np.float32

LN_EPS = 1e-5
LATENT = 256
HIDDEN = 256
D_NODE_IN = 128
D_EDGE_IN = 4
D_OUT = 128
STEPS = 6

NC = 8
CHUNK_REAL = 12500
CHUNK = 12800            # 100 windows * 128
NW = CHUNK // 128        # 100 receiver windows per core
TAB = NC * CHUNK         # 102400 padded global nodes
NBANK = 4
BANK = TAB // NBANK      # 25600 (nominal bank stride; int16 reaches 32767)
QUOTA = 128              # slots per (window, bank) = 1 chunk
WSLOTS = NBANK * QUOTA   # 512 slots per window
ES = NW * WSLOTS         # 51200 edge slots per core
NT = CHUNK // 512        # 25 node pieces of 512

# blob layouts: (name, tile_shape); column count = prod(shape[1:])
BFSPEC = [
    ('ident', (128, 128)),
    ('enc_n_w1', (128, 1, 256)), ('enc_n_b1', (128, 2)),
    ('enc_n_w2', (128, 2, 256)), ('enc_n_b2', (128, 2)),
    ('enc_e_b1', (128, 2)), ('enc_e_w2', (128, 2, 256)), ('enc_e_b2', (128, 2)),
    ('pe_w1', (128, 6 * STEPS, 256)), ('pe_b1', (128, 2 * STEPS)),
    ('pe_w2', (128, 2 * STEPS, 256)), ('pe_b2', (128, 2 * STEPS)),
    ('pn_w1', (128, 4 * STEPS, 256)), ('pn_b1', (128, 2 * STEPS)),
    ('pn_w2', (128, 2 * STEPS, 256)), ('pn_b2', (128, 2 * STEPS)),
    ('dec_w1', (128, 2, 256)), ('dec_b1', (128, 2)),
    ('dec_w2', (128, 2, 128)), ('dec_b2', (128, 1)),
]
F32SPEC = [
    ('iota', (128, 128)), ('rcvc', (128, ES // 128)),
    ('enc_n_s', (128, 2)), ('enc_n_o', (128, 2)),
    ('enc_e_s', (128, 2)), ('enc_e_o', (128, 2)),
    ('pe_s', (128, 2 * STEPS)), ('pe_o', (128, 2 * STEPS)),
    ('pn_s', (128, 2 * STEPS)), ('pn_o', (128, 2 * STEPS)),
]


def _blob_offsets(spec):
    off = {}
    c = 0
    for name, shape in spec:
        n = int(np.prod(shape[1:]))
        off[name] = (c, n, shape)
        c += n
    return off, c


BF_OFF, BF_COLS = _blob_offsets(BFSPEC)
BF_COLSP = -(-BF_COLS // 8) * 8          # pad so the flat blob splits 8 ways
WSL = 16 * BF_COLSP                      # per-core weight-slice elements
F32_OFF, F32_COLS = _blob_offsets(F32SPEC)


# ----------------------------------------------------------------------------
# host-side prep
# ----------------------------------------------------------------------------

def _wrap_idx(vals):
    """[n] int16 -> [16, n/16] wrapped: slot i at [i%16, i//16]. The required
    replication across the 8 groups of 16 partitions happens on device."""
    n = len(vals)
    a = np.asarray(vals, np.int16).reshape(n // 16, 16).T
    return np.ascontiguousarray(a)


def _prep_graph(senders, receivers, edge_features):
    s = np.asarray(senders, np.int64)
    r = np.asarray(receivers, np.int64)
    ef = np.asarray(edge_features, F32)
    E = len(s)
    ks = s // CHUNK_REAL
    ps = ks * CHUNK + (s - ks * CHUNK_REAL)      # padded global sender id
    kr = r // CHUNK_REAL
    rl = r - kr * CHUNK_REAL                     # local receiver id
    w = rl // 128
    pos = rl % 128
    b = ps // BANK

    # QUOTA=128: a few (core,window,bank) cells exceed 128 edges. int16 idx
    # reaches 32767 > BANK, so an edge with ps - (b-1)*BANK <= 32767 can be
    # served from the lower bank's gather base. Move overflow edges down.
    cell = ((kr * NW + w) * NBANK + b)
    cnt = np.bincount(cell, minlength=NC * NW * NBANK)
    for c in np.nonzero(cnt > QUOTA)[0][::-1]:  # higher banks first
        j = c % NBANK
        if j == 0:
            raise RuntimeError("bank-0 overflow: cannot borrow downward")
        over = cnt[c] - QUOTA
        cand = np.nonzero((cell == c) & (ps - (j - 1) * BANK <= 32767))[0]
        if len(cand) < over or cnt[c - 1] + over > QUOTA:
            raise RuntimeError("overflow not resolvable by bank borrowing")
        b[cand[:over]] -= 1
        cnt[c] -= over
        cnt[c - 1] += over
    group = ((kr * NW + w) * NBANK + b)
    order = np.argsort(group, kind='stable')
    g_sorted = group[order]
    # index within group
    uniq, starts, counts = np.unique(g_sorted, return_index=True, return_counts=True)
    if counts.max() > QUOTA:
        raise RuntimeError(f"(window,bank) overflow: {counts.max()} > {QUOTA}")
    within = np.arange(E) - np.repeat(starts, counts)
    slot = g_sorted * QUOTA + within              # global slot id: core*ES + local
    core = g_sorted // (NW * NBANK)
    local_slot = slot - core * (NW * NBANK * QUOTA)

    out = []
    for k in range(NC):
        sel = core == k
        e_ids = order[sel]
        sl = local_slot[sel]
        snd_rel = np.zeros(ES, np.int16)
        rcv_val = np.full(ES, 255.0, F32)
        efT = np.zeros((D_EDGE_IN, ES), F32)
        snd_rel[sl] = (ps[e_ids] - b[e_ids] * BANK).astype(np.int16)
        rcv_val[sl] = pos[e_ids].astype(F32)
        efT[:, sl] = ef[e_ids].T
        rcv_col = np.ascontiguousarray(rcv_val.reshape(ES // 128, 128).T)  # [128, 800]
        eid = np.full(ES, -1, np.int64)
        eid[sl] = e_ids
        out.append(dict(snd=_wrap_idx(snd_rel), rcvc=rcv_col,
                        efT=efT.astype(BF16), _eid=eid))
    return out


def _prep_weights(i):
    """Pack weights into the kernel layouts (bf16 unless noted)."""
    w = {}

    def w1pack(mat):  # [K, N] -> [128, K/128, N] lhsT chunks
        K, N = mat.shape
        assert K % 128 == 0
        return np.ascontiguousarray(
            np.asarray(mat, F32).reshape(K // 128, 128, N).transpose(1, 0, 2))

    def cols(vec_s):  # [S, 256] -> [128, S*2]
        v = np.asarray(vec_s, F32)
        if v.ndim == 1:
            v = v[None]
        S = v.shape[0]
        return np.ascontiguousarray(v.reshape(S, 2, 128).transpose(2, 0, 1).reshape(128, S * 2))

    w['enc_n_w1'] = w1pack(i['enc_node_w1'])                       # [128,1,256]
    w['enc_n_b1'] = cols(i['enc_node_b1'])
    w['enc_n_w2'] = w1pack(i['enc_node_w2'])                       # [128,2,256]
    w['enc_n_b2'] = cols(i['enc_node_b2'])
    w['enc_n_s'] = cols(i['enc_node_ln_s'])
    w['enc_n_o'] = cols(i['enc_node_ln_o'])
    w['enc_e_w1'] = np.asarray(i['enc_edge_w1'], F32)[:, None, :]  # [4,1,256]
    w['enc_e_b1'] = cols(i['enc_edge_b1'])
    w['enc_e_w2'] = w1pack(i['enc_edge_w2'])
    w['enc_e_b2'] = cols(i['enc_edge_b2'])
    w['enc_e_s'] = cols(i['enc_edge_ln_s'])
    w['enc_e_o'] = cols(i['enc_edge_ln_o'])

    w['pe_w1'] = np.concatenate([w1pack(i['pe_w1'][s]) for s in range(STEPS)], 1)  # [128,36,256]
    w['pe_b1'] = cols(i['pe_b1'])
    w['pe_w2'] = np.concatenate([w1pack(i['pe_w2'][s]) for s in range(STEPS)], 1)  # [128,12,256]
    w['pe_b2'] = cols(i['pe_b2'])
    w['pe_s'] = cols(i['pe_ln_s'])
    w['pe_o'] = cols(i['pe_ln_o'])
    w['pn_w1'] = np.concatenate([w1pack(i['pn_w1'][s]) for s in range(STEPS)], 1)  # [128,24,256]
    w['pn_b1'] = cols(i['pn_b1'])
    w['pn_w2'] = np.concatenate([w1pack(i['pn_w2'][s]) for s in range(STEPS)], 1)
    w['pn_b2'] = cols(i['pn_b2'])
    w['pn_s'] = cols(i['pn_ln_s'])
    w['pn_o'] = cols(i['pn_ln_o'])

    w['dec_w1'] = w1pack(i['dec_w1'])                              # [128,2,256]
    w['dec_b1'] = cols(i['dec_b1'])
    w['dec_w2'] = w1pack(np.asarray(i['dec_w2'], F32))             # [128,2,128]
    w['dec_b2'] = np.asarray(i['dec_b2'], F32)[:, None]            # [128,1]

    w['iota'] = np.tile(np.arange(128, dtype=F32)[None, :], (128, 1))
    w['ident'] = np.eye(128, dtype=F32)

    # pack into blobs (rcvc region of the f32 blob is per-core)
    blob_bf = np.zeros((128, BF_COLSP), BF16)
    for name, (c0, n, shape) in BF_OFF.items():
        blob_bf[:, c0:c0 + n] = w[name].reshape(128, n).astype(BF16)
    blob_f32 = np.zeros((128, F32_COLS), F32)
    for name, (c0, n, shape) in F32_OFF.items():
        if name == 'rcvc':
            continue
        blob_f32[:, c0:c0 + n] = w[name].reshape(128, n).astype(F32)
    return blob_bf, blob_f32, np.ascontiguousarray(w['enc_e_w1'].astype(BF16))


# ----------------------------------------------------------------------------
# program
# ----------------------------------------------------------------------------

def build_program(steps=STEPS, debug=False):
    dt = mybir.dt
    bf = dt.bfloat16
    f32 = dt.float32
    AF = mybir.ActivationFunctionType
    OP = mybir.AluOpType

    nc = bacc.Bacc(None, target_bir_lowering=False)

    def inp(name, shape, dtype=bf):
        return nc.dram_tensor(name, shape, dtype, kind="ExternalInput")

    t_wsl = inp("wsl", [WSL])            # this core's 1/8 of the weight blob
    t_nf = inp("nfT", [128, CHUNK])
    t_f32 = inp("blob_f32", [128, F32_COLS], f32)
    t_efT = inp("efT", [D_EDGE_IN, ES])
    t_snd = inp("snd16", [16, ES // 16], dt.int16)
    t_eew1 = inp("enc_e_w1", [D_EDGE_IN, 1, 256])

    t_out = nc.dram_tensor("outp", [CHUNK, D_OUT], bf, kind="ExternalOutput")

    # internal DRAM
    node_loc = nc.dram_tensor("node_loc", [CHUNK, LATENT], bf)
    cc_out = nc.dram_tensor("cc_out", [TAB, LATENT], bf, addr_space="Shared")
    edge_T = nc.dram_tensor("edge_T", [128, 2, ES], bf)
    nodeT = nc.dram_tensor("nodeT", [128, 2, CHUNK], bf)
    aggT = nc.dram_tensor("aggT", [128, 2, CHUNK], bf)
    C_d = nc.dram_tensor("C_d", [128, ES], bf)
    CT_d = nc.dram_tensor("CT_d", [128, ES], bf)
    snd_x = nc.dram_tensor("snd_x", [128, ES // 16], dt.int16)
    wsl_d = nc.dram_tensor("wsl_d", [WSL], bf)
    wfull = nc.dram_tensor("wfull", [NC * WSL], bf, addr_space="Shared")

    with tile.TileContext(nc) as tc:
        import contextlib
        stack = contextlib.ExitStack()
        wp = stack.enter_context(tc.tile_pool(name="wp", bufs=1))
        sb = stack.enter_context(tc.tile_pool(name="sb", bufs=2))
        ps2 = stack.enter_context(tc.tile_pool(name="ps2", bufs=2, space="PSUM"))
        ps1 = stack.enter_context(tc.tile_pool(name="ps1", bufs=1, space="PSUM"))

        # reassemble the weight blob: each core ships 1/8th, AllGather the rest
        wstage = wp.tile([128, WSL // 128], bf, tag="wstage")
        nc.sync.dma_start(out=wstage[:],
                          in_=t_wsl.ap().rearrange("(p c) -> p c", p=128))
        nc.sync.dma_start(out=wsl_d.ap().rearrange("(p c) -> p c", p=128),
                          in_=wstage[:])
        nc.gpsimd.collective_compute(
            "AllGather", mybir.AluOpType.bypass,
            ins=[wsl_d[:]], outs=[wfull[:]],
            replica_groups=[list(range(NC))])
        wfull2d = wfull.ap().rearrange("(p w) -> p w", p=128)

        wt = {}
        for name, (c0, ncols, shape) in list(BF_OFF.items()) + list(F32_OFF.items()):
            if name == 'rcvc':
                continue
            if name in dict(F32SPEC):
                t = wp.tile(list(shape), f32, tag=f"w_{name}")
                src = t_f32.ap()[:, c0:c0 + ncols]
            else:
                t = wp.tile(list(shape), bf, tag=f"w_{name}")
                src = wfull2d[:, c0:c0 + ncols]
            if len(shape) == 3:
                src = src.rearrange("p (k n) -> p k n", k=shape[1])
            nc.sync.dma_start(out=t[:], in_=src)
            wt[name] = t
        t = wp.tile([D_EDGE_IN, 1, 256], bf, tag="w_enc_e_w1")
        nc.sync.dma_start(out=t[:], in_=t_eew1[:])
        wt['enc_e_w1'] = t
        ones_t = wp.tile([128, 128], bf, tag="ones")
        nc.vector.memset(ones_t[:], 1.0)
        eps_t = wp.tile([128, 1], f32, tag="eps")
        nc.vector.memset(eps_t[:], LN_EPS)

        # expand snd16 [16, ES/16] -> snd_x [128, ES/16] (8 Q7-core replicas)
        s16 = wp.tile([16, ES // 16], dt.int16, tag="s16")
        nc.sync.dma_start(out=s16[:], in_=t_snd[:])
        for g in range(8):
            nc.sync.dma_start(out=snd_x.ap()[16 * g:16 * g + 16, :], in_=s16[:])

        def ln_half(o2, kb2, ks, ko, resid_ap, out_ap, fw):
            """PSUM o2 [128,2,FW] f32 -> LayerNorm over latent(=partitions,
            2 chunks) with scale/offset cols, optional residual, bf16 out."""
            o2sb = sb.tile([128, 2, 512], f32, tag="o2sb")
            o2bf = sb.tile([128, 2, 512], bf, tag="o2bf")
            sq = sb.tile([128, 2, 512], bf, tag="sq")
            for lc in range(2):
                nc.scalar.activation(out=o2sb[:, lc, :fw], in_=o2[:, lc, :fw],
                                     func=AF.Identity, bias=kb2[:, lc:lc + 1])
                nc.vector.tensor_copy(out=o2bf[:, lc, :fw], in_=o2sb[:, lc, :fw])
                nc.scalar.activation(out=sq[:, lc, :fw], in_=o2sb[:, lc, :fw],
                                     func=AF.Square)
            S = ps1.tile([128, 512], f32, tag="S")
            nc.tensor.matmul(S[:, :fw], lhsT=ones_t[:], rhs=o2bf[:, 0, :fw],
                             start=True, stop=False)
            nc.tensor.matmul(S[:, :fw], lhsT=ones_t[:], rhs=o2bf[:, 1, :fw],
                             start=False, stop=True)
            S2 = ps1.tile([128, 512], f32, tag="S2")
            nc.tensor.matmul(S2[:, :fw], lhsT=ones_t[:], rhs=sq[:, 0, :fw],
                             start=True, stop=False)
            nc.tensor.matmul(S2[:, :fw], lhsT=ones_t[:], rhs=sq[:, 1, :fw],
                             start=False, stop=True)
            m = sb.tile([128, 512], f32, tag="st_m")
            v = sb.tile([128, 512], f32, tag="st_v")
            nc.vector.tensor_scalar(out=m[:, :fw], in0=S[:, :fw], scalar1=1.0 / LATENT,
                                    scalar2=None, op0=OP.mult)
            nc.vector.tensor_scalar(out=v[:, :fw], in0=S2[:, :fw], scalar1=1.0 / LATENT,
                                    scalar2=None, op0=OP.mult)
            msq = sb.tile([128, 512], f32, tag="st_m2")
            nc.vector.tensor_tensor(out=msq[:, :fw], in0=m[:, :fw], in1=m[:, :fw],
                                    op=OP.mult)
            nc.vector.tensor_tensor(out=v[:, :fw], in0=v[:, :fw], in1=msq[:, :fw],
                                    op=OP.subtract)
            inv = sb.tile([128, 512], f32, tag="st_i")
            nc.scalar.activation(out=inv[:, :fw], in_=v[:, :fw], func=AF.Sqrt,
                                 bias=eps_t[:])
            nc.vector.reciprocal(inv[:, :fw], inv[:, :fw])
            b = sb.tile([128, 512], f32, tag="st_b")
            nc.vector.tensor_tensor(out=b[:, :fw], in0=m[:, :fw], in1=inv[:, :fw],
                                    op=OP.mult)
            for lc in range(2):
                y = sb.tile([128, 512], f32, tag="st_y")
                nc.vector.tensor_tensor(out=y[:, :fw], in0=o2sb[:, lc, :fw],
                                        in1=inv[:, :fw], op=OP.mult)
                nc.vector.tensor_tensor(out=y[:, :fw], in0=y[:, :fw], in1=b[:, :fw],
                                        op=OP.subtract)
                z = sb.tile([128, 512], f32, tag="st_z")
                nc.vector.tensor_scalar(out=z[:, :fw], in0=y[:, :fw],
                                        scalar1=ks[:, lc:lc + 1],
                                        scalar2=ko[:, lc:lc + 1],
                                        op0=OP.mult, op1=OP.add)
                if resid_ap is not None:
                    nc.vector.tensor_tensor(out=out_ap[lc], in0=z[:, :fw],
                                            in1=resid_ap[lc], op=OP.add)
                else:
                    nc.vector.tensor_copy(out=out_ap[lc], in_=z[:, :fw])

        # ---------------- C build ----------------
        NCH = WSLOTS // 128  # chunks per window
        with tc.For_i(0, NW, name="cbuild") as w:
            rc = sb.tile([128, NCH], f32, tag="rc")
            nc.sync.dma_start(out=rc[:], in_=t_f32[:, ds(w * NCH + F32_OFF['rcvc'][0], NCH)])
            cw = sb.tile([128, WSLOTS], bf, tag="cw")
            ctw = sb.tile([128, WSLOTS], bf, tag="ctw")
            for j in range(NCH):
                nc.vector.tensor_scalar(out=cw[:, 128 * j:128 * j + 128],
                                        in0=wt['iota'][:], scalar1=rc[:, j:j + 1],
                                        scalar2=None, op0=OP.is_equal)
                tp = ps2.tile([128, 128], bf, tag="sm")
                nc.tensor.transpose(out=tp[:], in_=cw[:, 128 * j:128 * j + 128],
                                    identity=wt['ident'][:])
                if j % 2 == 0:
                    nc.vector.tensor_copy(out=ctw[:, 128 * j:128 * j + 128], in_=tp[:])
                else:
                    nc.scalar.activation(out=ctw[:, 128 * j:128 * j + 128], in_=tp[:],
                                         func=AF.Copy)
            nc.sync.dma_start(out=C_d[:, ts(w, WSLOTS)], in_=cw[:])
            nc.sync.dma_start(out=CT_d[:, ts(w, WSLOTS)], in_=ctw[:])

        # ---------------- node encoder ----------------
        def mlp_tail(htb, wname, bname, sname, oname, sidx, resid, outf, t, dram,
                     fw=512):
            """L2 (+bias) + LN + optional residual + store to dram[:, :, ts]."""
            o2 = ps1.tile([128, 2, 512], f32, tag="o2")
            for lc in range(2):
                for kc in range(2):
                    nc.tensor.matmul(o2[:, lc, :fw],
                                     lhsT=wt[wname][:, 2 * sidx + kc, 128 * lc:128 * lc + 128],
                                     rhs=htb[:, kc, :fw],
                                     start=(kc == 0), stop=(kc == 1))
            newt = sb.tile([128, 2, 512], bf, tag="newt")
            resid_ap = None
            if resid is not None:
                resid_ap = [resid[:, lc, :fw] for lc in range(2)]
            ln_half(o2, wt[bname][:, 2 * sidx:2 * sidx + 2],
                    wt[sname][:, 2 * sidx:2 * sidx + 2],
                    wt[oname][:, 2 * sidx:2 * sidx + 2],
                    resid_ap, [newt[:, lc, :fw] for lc in range(2)], fw)
            nc.sync.dma_start(out=dram[:, :, ts(t, fw)], in_=newt[:, :, :fw])
            return newt

        def store_normal(newt, t, dram, nch=4, dtile=LATENT):
            """Transpose [128,2,512]T tile -> normal rows, write dram rows."""
            nn = sb.tile([128, 4, 256], dram.dtype, tag="nn4")
            for c in range(nch):
                for lc in range(2):
                    tp = ps2.tile([128, 128], bf, tag="sm")
                    nc.tensor.transpose(out=tp[:], in_=newt[:, lc, 128 * c:128 * c + 128],
                                        identity=wt['ident'][:])
                    if (c + lc) % 2 == 0:
                        nc.vector.tensor_copy(out=nn[:, c, 128 * lc:128 * lc + 128], in_=tp[:])
                    else:
                        nc.scalar.activation(out=nn[:, c, 128 * lc:128 * lc + 128],
                                             in_=tp[:], func=AF.Copy)
            nc.sync.dma_start(
                out=dram.ap()[ts(t, 512)].rearrange("(c p) d -> p c d", p=128),
                in_=nn[:, :nch, :])

        with tc.For_i(0, NT, name="enc_n") as t:
            nf = sb.tile([128, 512], bf, tag="nf")
            nc.sync.dma_start(out=nf[:], in_=t_nf[:, ts(t, 512)])
            htb = sb.tile([128, 2, 512], bf, tag="htb")
            for m in range(2):
                hp = ps1.tile([128, 512], f32, tag="hp")
                nc.tensor.matmul(hp[:], lhsT=wt['enc_n_w1'][:, 0, 128 * m:128 * m + 128],
                                 rhs=nf[:], start=True, stop=True)
                nc.scalar.activation(out=htb[:, m, :], in_=hp[:], func=AF.Silu,
                                     bias=wt['enc_n_b1'][:, m:m + 1])
            newt = mlp_tail(htb, 'enc_n_w2', 'enc_n_b2', 'enc_n_s', 'enc_n_o', 0,
                            None, None, t, nodeT)
            store_normal(newt, t, node_loc)

        # ---------------- edge encoder ----------------
        with tc.For_i(0, NW, name="enc_e") as w:
            ef = sb.tile([D_EDGE_IN, WSLOTS], bf, tag="ef")
            nc.sync.dma_start(out=ef[:], in_=t_efT[:, ts(w, WSLOTS)])
            htb = sb.tile([128, 2, 512], bf, tag="htb")
            for m in range(2):
                hp = ps1.tile([128, 512], f32, tag="hp")
                nc.tensor.matmul(hp[:], lhsT=wt['enc_e_w1'][:, 0, 128 * m:128 * m + 128],
                                 rhs=ef[:], start=True, stop=True)
                nc.scalar.activation(out=htb[:, m, :], in_=hp[:], func=AF.Silu,
                                     bias=wt['enc_e_b1'][:, m:m + 1])
            o2 = ps1.tile([128, 2, 512], f32, tag="o2")
            for lc in range(2):
                for kc in range(2):
                    nc.tensor.matmul(o2[:, lc, :],
                                     lhsT=wt['enc_e_w2'][:, kc, 128 * lc:128 * lc + 128],
                                     rhs=htb[:, kc, :], start=(kc == 0), stop=(kc == 1))
            newt = sb.tile([128, 2, 512], bf, tag="newt")
            ln_half(o2, wt['enc_e_b2'][:, 0:2], wt['enc_e_s'][:, 0:2],
                    wt['enc_e_o'][:, 0:2], None,
                    [newt[:, lc, :] for lc in range(2)], 512)
            nc.sync.dma_start(out=edge_T[:, :, ts(w, WSLOTS)], in_=newt[:])

        # ---------------- message passing ----------------
        for s in range(steps):
            nc.gpsimd.collective_compute(
                "AllGather", mybir.AluOpType.bypass,
                ins=[node_loc[:]], outs=[cc_out[:]],
                replica_groups=[list(range(NC))])

            with tc.For_i(0, NW, name=f"edge{s}") as w:
                xe = sb.tile([128, 2, WSLOTS], bf, tag="xe")
                nc.sync.dma_start(out=xe[:], in_=edge_T[:, :, ts(w, WSLOTS)])
                idxt = sb.tile([128, WSLOTS // 16], mybir.dt.int16, tag="idxt")
                nc.sync.dma_start(out=idxt[:], in_=snd_x.ap()[:, ts(w, WSLOTS // 16)])
                xst = sb.tile([128, NBANK, 2, QUOTA], bf, tag="xst")
                for b in range(NBANK):
                    nc.gpsimd.dma_gather(
                        out_ap=xst[:, b, :, :],
                        in_ap=cc_out.ap()[BANK * b:min(BANK * b + 32768, TAB)],
                        idxs_ap=idxt[:, (QUOTA // 16) * b:(QUOTA // 16) * (b + 1)],
                        num_idxs=QUOTA, num_idxs_reg=QUOTA, elem_size=LATENT,
                        transpose=True)
                xs = sb.tile([128, 2, WSLOTS], bf, tag="xs")
                for b in range(NBANK):
                    for lc in range(2):
                        if (b + lc) % 2 == 0:
                            nc.vector.tensor_copy(out=xs[:, lc, QUOTA * b:QUOTA * b + QUOTA],
                                                  in_=xst[:, b, lc, :])
                        else:
                            nc.scalar.activation(out=xs[:, lc, QUOTA * b:QUOTA * b + QUOTA],
                                                 in_=xst[:, b, lc, :], func=AF.Copy)
                ndw = sb.tile([128, LATENT], bf, tag="ndw")
                nc.sync.dma_start(out=ndw[:], in_=node_loc.ap()[ts(w, 128)])
                ct = sb.tile([128, WSLOTS], bf, tag="ct")
                nc.sync.dma_start(out=ct[:], in_=CT_d[:, ts(w, WSLOTS)])
                cc = sb.tile([128, WSLOTS], bf, tag="cc")
                nc.sync.dma_start(out=cc[:], in_=C_d[:, ts(w, WSLOTS)])
                xr = sb.tile([128, 2, WSLOTS], bf, tag="xr")
                for j in range(NCH):
                    for lc in range(2):
                        xp = ps2.tile([128, 128], f32, tag="sm")
                        nc.tensor.matmul(xp[:], lhsT=ndw[:, 128 * lc:128 * lc + 128],
                                         rhs=ct[:, 128 * j:128 * j + 128],
                                         start=True, stop=True)
                        if (j + lc) % 2 == 0:
                            nc.vector.tensor_copy(out=xr[:, lc, 128 * j:128 * j + 128], in_=xp[:])
                        else:
                            nc.scalar.activation(out=xr[:, lc, 128 * j:128 * j + 128],
                                                 in_=xp[:], func=AF.Copy)

                newe = sb.tile([128, 2, WSLOTS], bf, tag="newe")
                htb = sb.tile([128, 2, 512], bf, tag="htb")
                for m in range(2):
                    hp = ps1.tile([128, 512], f32, tag="hp")
                    first = True
                    for src, kbase in ((xe, 0), (xs, 2), (xr, 4)):
                        for kc in range(2):
                            ci = kbase + kc
                            nc.tensor.matmul(
                                hp[:], lhsT=wt['pe_w1'][:, 6 * s + ci, 128 * m:128 * m + 128],
                                rhs=src[:, kc, :], start=first, stop=(ci == 5))
                            first = False
                    nc.scalar.activation(out=htb[:, m, :], in_=hp[:], func=AF.Silu,
                                         bias=wt['pe_b1'][:, 2 * s + m:2 * s + m + 1])
                o2 = ps1.tile([128, 2, 512], f32, tag="o2")
                for lc in range(2):
                    for kc in range(2):
                        nc.tensor.matmul(o2[:, lc, :],
                                         lhsT=wt['pe_w2'][:, 2 * s + kc, 128 * lc:128 * lc + 128],
                                         rhs=htb[:, kc, :], start=(kc == 0), stop=(kc == 1))
                ln_half(o2, wt['pe_b2'][:, 2 * s:2 * s + 2],
                        wt['pe_s'][:, 2 * s:2 * s + 2], wt['pe_o'][:, 2 * s:2 * s + 2],
                        [xe[:, lc, :] for lc in range(2)],
                        [newe[:, lc, :] for lc in range(2)], 512)
                nc.sync.dma_start(out=edge_T[:, :, ts(w, WSLOTS)], in_=newe[:])

                # aggregation: aggT[:, lc, w*128:] = sum_j newe_j^T @ C_j
                # (the two lc chains share one PSUM bank: keep each chain
                # contiguous -- a group's first matmul clears the whole
                # bank's has_written bits)
                agg = ps1.tile([128, 2, 128], f32, tag="agg")
                nn8 = sb.tile([128, NCH, 256], bf, tag="nn8")
                for j in range(NCH):
                    for lc in range(2):
                        tp = ps2.tile([128, 128], bf, tag="sm")
                        nc.tensor.transpose(out=tp[:], in_=newe[:, lc, 128 * j:128 * j + 128],
                                            identity=wt['ident'][:])
                        if lc == 0:
                            nc.vector.tensor_copy(out=nn8[:, j, 128 * lc:128 * lc + 128], in_=tp[:])
                        else:
                            nc.scalar.activation(out=nn8[:, j, 128 * lc:128 * lc + 128],
                                                 in_=tp[:], func=AF.Copy)
                for lc in range(2):
                    for j in range(NCH):
                        nc.tensor.matmul(agg[:, lc, :], lhsT=nn8[:, j, 128 * lc:128 * lc + 128],
                                         rhs=cc[:, 128 * j:128 * j + 128],
                                         start=(j == 0), stop=(j == NCH - 1))
                agsb = sb.tile([128, 2, 128], bf, tag="agsb")
                nc.vector.tensor_copy(out=agsb[:, 0, :], in_=agg[:, 0, :])
                nc.scalar.activation(out=agsb[:, 1, :], in_=agg[:, 1, :], func=AF.Copy)
                nc.sync.dma_start(out=aggT[:, :, ts(w, 128)], in_=agsb[:])

            with tc.For_i(0, NT, name=f"node{s}") as t:
                ntw = sb.tile([128, 2, 512], bf, tag="ntw")
                nc.sync.dma_start(out=ntw[:], in_=nodeT[:, :, ts(t, 512)])
                agw = sb.tile([128, 2, 512], bf, tag="agw")
                nc.sync.dma_start(out=agw[:], in_=aggT[:, :, ts(t, 512)])
                htb = sb.tile([128, 2, 512], bf, tag="htb")
                for m in range(2):
                    hp = ps1.tile([128, 512], f32, tag="hp")
                    first = True
                    for src, kbase in ((ntw, 0), (agw, 2)):
                        for kc in range(2):
                            ci = kbase + kc
                            nc.tensor.matmul(
                                hp[:], lhsT=wt['pn_w1'][:, 4 * s + ci, 128 * m:128 * m + 128],
                                rhs=src[:, kc, :], start=first, stop=(ci == 3))
                            first = False
                    nc.scalar.activation(out=htb[:, m, :], in_=hp[:], func=AF.Silu,
                                         bias=wt['pn_b1'][:, 2 * s + m:2 * s + m + 1])
                newt = mlp_tail(htb, 'pn_w2', 'pn_b2', 'pn_s', 'pn_o', s, ntw,
                                None, t, nodeT)
                store_normal(newt, t, node_loc)

        # ---------------- decoder ----------------
        with tc.For_i(0, NT, name="dec") as t:
            ntw = sb.tile([128, 2, 512], bf, tag="ntw")
            nc.sync.dma_start(out=ntw[:], in_=nodeT[:, :, ts(t, 512)])
            htb = sb.tile([128, 2, 512], bf, tag="htb")
            for m in range(2):
                hp = ps1.tile([128, 512], f32, tag="hp")
                for kc in range(2):
                    nc.tensor.matmul(hp[:], lhsT=wt['dec_w1'][:, kc, 128 * m:128 * m + 128],
                                     rhs=ntw[:, kc, :], start=(kc == 0), stop=(kc == 1))
                nc.scalar.activation(out=htb[:, m, :], in_=hp[:], func=AF.Silu,
                                     bias=wt['dec_b1'][:, m:m + 1])
            od = ps1.tile([128, 512], f32, tag="hp")
            for kc in range(2):
                nc.tensor.matmul(od[:], lhsT=wt['dec_w2'][:, kc, :],
                                 rhs=htb[:, kc, :], start=(kc == 0), stop=(kc == 1))
            odsb = sb.tile([128, 512], bf, tag="odsb")
            nc.scalar.activation(out=odsb[:], in_=od[:], func=AF.Identity,
                                 bias=wt['dec_b2'][:, 0:1])
            ow = sb.tile([128, 4, D_OUT], bf, tag="ow")
            for c in range(4):
                tp = ps2.tile([128, 128], bf, tag="sm")
                nc.tensor.transpose(out=tp[:], in_=odsb[:, 128 * c:128 * c + 128],
                                    identity=wt['ident'][:])
                if c % 2 == 0:
                    nc.vector.tensor_copy(out=ow[:, c, :], in_=tp[:])
                else:
                    nc.scalar.activation(out=ow[:, c, :], in_=tp[:], func=AF.Copy)
            nc.sync.dma_start(
                out=t_out.ap()[ts(t, 512)].rearrange("(c p) d -> p c d", p=128),
                in_=ow[:])

        if debug:
            for name, src in [("d_nodeT", nodeT), ("d_edgeT", edge_T),
                              ("d_aggT", aggT), ("d_C", C_d), ("d_CT", CT_d),
                              ("d_cc", cc_out), ("d_nloc", node_loc)]:
                dst = nc.dram_tensor(name, list(src.shape), bf,
                                     kind="ExternalOutput")
                tot = int(np.prod(src.shape))
                nchunk = -(-tot // (128 * 4096))
                while tot % nchunk or (tot // nchunk) % 128:
                    nchunk += 1
                step_e = tot // nchunk
                for i in range(nchunk):
                    tmp = sb.tile([128, step_e // 128], bf, tag="dbg")
                    nc.sync.dma_start(
                        out=tmp[:],
                        in_=src.ap().rearrange(
                            *(["a b c -> (a b c)"] if len(src.shape) == 3
                              else ["a b -> (a b)"]))[i * step_e:(i + 1) * step_e]
                        .rearrange("(p c) -> p c", p=128))
                    nc.sync.dma_start(
                        out=dst.ap().rearrange(
                            *(["a b c -> (a b c)"] if len(src.shape) == 3
                              else ["a b -> (a b)"]))[i * step_e:(i + 1) * step_e]
                        .rearrange("(p c) -> p c", p=128),
                        in_=tmp[:])
        stack.close()
    nc.finalize()
    return nc


# ----------------------------------------------------------------------------
# host wrapper
# ----------------------------------------------------------------------------

LAST_EXEC_NS = None
_PROG = None


def kernel(**inputs):
    global LAST_EXEC_NS, _PROG
    import os, time
    try:
        import jax
        jax.config.update('jax_compilation_cache_dir', '/tmp/gnn_jax_cache')
        jax.config.update('jax_persistent_cache_min_compile_time_secs', 0.0)
        jax.config.update('jax_persistent_cache_min_entry_size_bytes', -1)
    except Exception:
        pass
    inputs = {k: np.asarray(v) for k, v in inputs.items()}
    n_nodes = inputs['node_features'].shape[0]

    t0 = time.time()
    graph = _prep_graph(inputs['senders'], inputs['receivers'],
                        inputs['edge_features'])
    blob_bf, blob_f32, eew1 = _prep_weights(inputs)
    wflat = np.ascontiguousarray(blob_bf).reshape(-1)
    nf = np.asarray(inputs['node_features'], F32)
    in_maps = []
    c0_rc = F32_OFF['rcvc'][0]
    for k in range(NC):
        nfT = np.zeros((128, CHUNK), BF16)
        real = min(CHUNK_REAL, n_nodes - k * CHUNK_REAL)
        nfT[:, :real] = nf[k * CHUNK_REAL:k * CHUNK_REAL + real].T.astype(BF16)
        bf32 = blob_f32.copy()
        bf32[:, c0_rc:c0_rc + ES // 128] = graph[k]['rcvc']
        in_maps.append(dict(wsl=wflat[k * WSL:(k + 1) * WSL], nfT=nfT,
                            blob_f32=bf32, enc_e_w1=eew1,
                            efT=graph[k]['efT'], snd16=graph[k]['snd']))
    print(f"[kernel] host prep {time.time()-t0:.1f}s", flush=True)

    t0 = time.time()
    if _PROG is None:
        _PROG = build_program()
    print(f"[kernel] build {time.time()-t0:.1f}s", flush=True)

    from concourse.bass_utils import run_bass_kernel_spmd
    t0 = time.time()
    res = run_bass_kernel_spmd(_PROG, in_maps, core_ids=list(range(NC)))
    t1 = time.time()
    print(f"[kernel] run {t1-t0:.1f}s", flush=True)
    LAST_EXEC_NS = res.exec_time_ns
    if os.environ.get("GNN_TIME2"):
        t0 = time.time()
        res = run_bass_kernel_spmd(_PROG, in_maps, core_ids=list(range(NC)))
        t2 = time.time()
        print(f"[kernel] warm run {t2-t0:.1f}s", flush=True)
        LAST_EXEC_NS = int((t2 - t0) * 1e9)

    out = np.empty((n_nodes, D_OUT), np.float32)
    for k in range(NC):
        real = min(CHUNK_REAL, n_nodes - k * CHUNK_REAL)
        out[k * CHUNK_REAL:k * CHUNK_REAL + real] = \
            np.asarray(res.results[k]['outp'][:real], np.float32)
    return out
